# revision 9
# baseline (speedup 1.0000x reference)
"""BERT-base forward on 8 Trainium2 NeuronCores.

Strategy: pure data parallelism over the batch (B=16 -> 2 sequences per
core), weights replicated, zero collectives.

v6 redesign (from v5's trace: ACT 52%/2.4ms + DVE 36%/1.66ms serialized
against PE 85%/3.9ms, 29% of the span HAM-throttled at half clock):

* k-major attention: scores are computed transposed ([k, q]) so the
  softmax probabilities feed the ctx matmul directly -- the 384 PE
  transpose-matmuls + 96 psum copies per layer of v5 are gone.  The
  softmax denominator comes for free as row 64 of the ctx matmul
  (V carries an appended ones column, M=65); normalization happens
  after ctx via DVE-recip -> gpsimd partition-broadcast -> DVE mult.
* token->feature layout changes (xtok -> xT) run on the DMA xbar
  transpose engine instead of PE matmul + ACT copy.
* PSUM evacuations moved from ACT to DVE; ACT keeps exp/gelu/sqrt and
  the bf16 xtok copies only.
* 1/sqrt(DH) folded into the exp scale; bv folded into the attn-out
  row bias host-side (brow_o = bv @ Wo + bo); mask folded into the
  exp per-partition bias (k-major makes the mask per-partition).
* single rotating weight-chunk pool ([128,768] bf16 x 24 bufs) streams
  Wq/Wk/Wv/Wo/W1/W2 and prefetches across phases.

Layout conventions per core (P=128 partitions):
  tokens NT=1024 (2 seqs x 512), token chunk t in [0,8)
  token-major  [128 tokens, H]   - residual stream, layernorm
  feature-major xT [128, t, hc, 128] - matmul lhsT/rhs operands
  matmul computes out = lhsT.T @ rhs (contraction along partitions)
"""

import numpy as np
import ml_dtypes

V, H, L, NH, I, S = 30522, 768, 12, 12, 3072, 512
B_FULL, NCORES, B_LOC = 16, 8, 2
DH = H // NH                      # 64
P = 128
NT = B_LOC * S                    # 1024 tokens per core
TC = NT // P                      # 8 token chunks
HC = H // P                       # 6 feature chunks
IC = I // P                       # 24 ffn chunks
SC = S // P                       # 4 chunks per sequence
EPS = 1e-12
INV_SQRT_DH = 1.0 / 8.0

_BF16 = ml_dtypes.bfloat16


# --------------------------------------------------------------------------
# device kernel builder
# --------------------------------------------------------------------------

def build(layers=L, taps=None, with_mask=False, with_brow=False):
    import concourse.bass as bass
    import concourse.mybir as mybir
    import concourse.tile as tile
    from concourse import bacc
    from contextlib import ExitStack

    dt = mybir.dt
    AF = mybir.ActivationFunctionType
    OP = mybir.AluOpType

    nc = bacc.Bacc("TRN2", target_bir_lowering=False, debug=False,
                   num_devices=NCORES)

    # ---- DRAM inputs (per core) ----
    wrows = nc.dram_tensor("wrows", [NT, H], dt.bfloat16, kind="ExternalInput")
    trows = nc.dram_tensor("trows", [NT, H], dt.bfloat16, kind="ExternalInput")
    pemb = nc.dram_tensor("pemb", [S, H], dt.float32, kind="ExternalInput")
    # mask as per-partition columns: extmc[p, s*SC+kc] = mask bias of
    # k-token s*512 + kc*128 + p (k-major scores make the mask per-partition)
    extmc = nc.dram_tensor("extmc", [P, B_LOC * SC], dt.float32,
                           kind="ExternalInput")
    dWq = nc.dram_tensor("Wq", [L, H, H], dt.bfloat16, kind="ExternalInput")
    dWk = nc.dram_tensor("Wk", [L, H, H], dt.bfloat16, kind="ExternalInput")
    dWv = nc.dram_tensor("Wv", [L, H, H], dt.bfloat16, kind="ExternalInput")
    dWo = nc.dram_tensor("Wo", [L, H, H], dt.bfloat16, kind="ExternalInput")
    dW1 = nc.dram_tensor("W1", [L, H, I], dt.bfloat16, kind="ExternalInput")
    dW2 = nc.dram_tensor("W2", [L, I, H], dt.bfloat16, kind="ExternalInput")
    dbq = nc.dram_tensor("bq", [L, H], dt.float32, kind="ExternalInput")
    dbk = nc.dram_tensor("bk", [L, H], dt.float32, kind="ExternalInput")
    db1 = nc.dram_tensor("b1", [L, I], dt.float32, kind="ExternalInput")
    # free-dim biases (added via K=1 rank-1 matmuls): rows [bv@Wo+bo, b2]
    dbrow = nc.dram_tensor("brow", [L, 1, 2 * H], dt.bfloat16, kind="ExternalInput")
    out = nc.dram_tensor("out", [NT, H], dt.float32, kind="ExternalOutput")

    f32, bf16 = dt.float32, dt.bfloat16

    def tap(name, tiles):
        if taps is None:
            return
        sh0 = list(tiles[0].shape)
        d = nc.dram_tensor(f"tap_{name}", [len(tiles)] + sh0,
                           tiles[0].dtype, kind="ExternalOutput")
        for i, t in enumerate(tiles):
            nc.sync.dma_start(d.ap()[i], t[:])
        taps[name] = d

    with tile.TileContext(nc) as tc_, ExitStack() as top:
        tc = tc_

        # ---- constants & persistent activation tiles ----
        pers = top.enter_context(tc.tile_pool(name="pers", bufs=1))
        ones1 = pers.tile([1, P], bf16, name="ones1")
        nc.vector.memset(ones1[:], 1.0)
        eps_t = pers.tile([P, 1], f32, name="eps_t")
        nc.vector.memset(eps_t[:], EPS)
        extm_sb = pers.tile([P, B_LOC * SC], f32, name="extm_sb")
        nc.sync.dma_start(extm_sb[:], extmc.ap())

        curA = [pers.tile([P, H], f32, name=f"curA{t}") for t in range(TC)]
        curB = [pers.tile([P, H], f32, name=f"curB{t}") for t in range(TC)]
        xtok = [pers.tile([P, H], bf16, name=f"xtok{t}") for t in range(TC)]
        # feature-major activations: xT[p, t, hc, j] = x[token t*128+j,
        # feature hc*128+p], written by DMA xbar transpose per t-chunk
        xT = pers.tile([P, TC, HC, P], bf16, name="xT")

        small = top.enter_context(tc.tile_pool(name="small", bufs=6))
        psum = top.enter_context(tc.tile_pool(name="psum", space="PSUM", bufs=1))
        # psum tags: "a" scores/QK/FFN1 [128,512]x2; "c" ctx [128,512]x2;
        # "b" V/attn-out/FFN2 [128,1024]x2 (768 used).  2+2+4 banks = 8.

        def rhs_x(nf, hi):
            """feature-major rhs [128, 512] for sequence-half nf, chunk hi."""
            return xT[:, 4 * nf:4 * nf + 4, hi, :]

        # ---------------- helpers ----------------
        def ln_store(src_ap, res_ap, dst, tcid, last=False, out_f32=None):
            """dst = layernorm(src + res); also writes bf16 copy to xtok[tcid]
            unless last (then DMAs fp32 to out_f32)."""
            s1 = small.tile([P, 1], f32, tag="s1")
            nc.vector.scalar_tensor_tensor(
                out=dst[:], in0=src_ap, scalar=0.0, in1=res_ap,
                op0=OP.add, op1=OP.add, accum_out=s1[:])
            u = small.tile([P, 1], f32, tag="u")
            nc.vector.tensor_scalar(out=u[:], in0=s1[:], scalar1=1.0 / H,
                                    scalar2=None, op0=OP.mult)
            junk = small.tile([P, H], f32, tag="junk", bufs=2)
            s2 = small.tile([P, 1], f32, tag="s2")
            nc.vector.scalar_tensor_tensor(
                out=junk[:], in0=dst[:], scalar=u[:], in1=dst[:],
                op0=OP.subtract, op1=OP.mult, accum_out=s2[:])
            sd = small.tile([P, 1], f32, tag="sd")
            nc.scalar.activation(sd[:], s2[:], AF.Sqrt, bias=eps_t[:], scale=1.0 / H)
            rstd = small.tile([P, 1], f32, tag="rstd")
            nc.vector.reciprocal(rstd[:], sd[:])
            nc.vector.tensor_scalar(out=dst[:], in0=dst[:], scalar1=u[:],
                                    scalar2=rstd[:], op0=OP.subtract, op1=OP.mult)
            if last:
                nc.scalar.dma_start(out_f32, dst[:])
            else:
                nc.scalar.copy(xtok[tcid][:], dst[:])

        def transpose_t(t):
            """xtok[t] (token-major bf16) -> xT[:, t] via DMA xbar.

            Dispatched on the scalar-engine HWDGE queue so transposes never
            queue behind weight loads on the sync queue (both are FIFO)."""
            nc.scalar.dma_start_transpose(xT[:, t], xtok[t][:])

        # ---- embedding: gather + add + LN ----
        with ExitStack() as emb_scope:
            ep = emb_scope.enter_context(tc.tile_pool(name="emb", bufs=1))
            wg = ep.tile([P, TC, H], bf16, name="wg")
            tg = ep.tile([P, TC, H], bf16, name="tg")
            nc.sync.dma_start(wg[:], wrows.ap().rearrange("(c p) h -> p c h", p=P))
            nc.sync.dma_start(tg[:], trows.ap().rearrange("(c p) h -> p c h", p=P))
            pos = ep.tile([P, SC, H], f32, name="pos")
            nc.sync.dma_start(pos[:], pemb.ap().rearrange("(c p) h -> p c h", p=P))
            for t in range(TC):
                tmp = ep.tile([P, H], f32, tag="etmp", bufs=2, name="etmp")
                nc.vector.tensor_tensor(out=tmp[:], in0=tg[:, t],
                                        in1=pos[:, t % SC], op=OP.add)
                ln_store(wg[:, t], tmp[:], curA[t], t)
                transpose_t(t)
            tap("emb", curA)

        # ---- transformer layers ----
        for l in range(layers):
            with ExitStack() as ls:
                wp = ls.enter_context(tc.tile_pool(name=f"bias{l}", bufs=1))
                bq_t = wp.tile([P, HC], f32, name=f"bq{l}")
                bk_t = wp.tile([P, HC], f32, name=f"bk{l}")
                b1_t = wp.tile([P, IC], f32, name=f"b1{l}")
                nc.sync.dma_start(bq_t[:], dbq.ap()[l].rearrange("(c p) -> p c", p=P))
                nc.sync.dma_start(bk_t[:], dbk.ap()[l].rearrange("(c p) -> p c", p=P))
                nc.sync.dma_start(b1_t[:], db1.ap()[l].rearrange("(c p) -> p c", p=P))
                brow_t = wp.tile([1, 2 * H], bf16, name=f"brow{l}")
                nc.sync.dma_start(brow_t[:], dbrow.ap()[l])

                # single rotating pool for all weight chunks of this layer
                wpool = ls.enter_context(tc.tile_pool(name=f"w{l}", bufs=24))

                def wload(dW, r0, c0, cols=H):
                    w = wpool.tile([P, H], bf16, tag="wc")
                    nc.sync.dma_start(w[:, :cols],
                                      dW.ap()[l, r0:r0 + P, c0:c0 + cols])
                    return w

                cur, nxt = (curA, curB)

                with ExitStack() as attn_scope:
                    ap_ = attn_scope.enter_context(
                        tc.tile_pool(name=f"attn{l}", bufs=1))
                    QT = [ap_.tile([P, NT], bf16, name=f"QT{l}_{h}") for h in range(HC)]
                    KT = [ap_.tile([P, NT], bf16, name=f"KT{l}_{h}") for h in range(HC)]
                    # V with an appended ones column per head (65 cols/head)
                    Vt = [ap_.tile([P, NH, DH + 1], bf16, name=f"V{l}_{t}")
                          for t in range(TC)]
                    ctxT = [ap_.tile([P, NT], bf16, name=f"cT{l}_{h}") for h in range(HC)]

                    wv_c = [wload(dWv, h * P, 0) for h in range(HC)]
                    wq_c = [wload(dWq, h * P, 0) for h in range(HC)]
                    wk_c = [wload(dWk, h * P, 0) for h in range(HC)]
                    wo_c = [wload(dWo, h * P, 0) for h in range(HC)]

                    def qk_group(dstT, wch, bt, ho, nf):
                        ps = psum.tile([P, S], f32, tag="a", bufs=2, name="psqk")
                        for hi in range(HC):
                            nc.tensor.matmul(
                                ps[:], lhsT=wch[hi][:, ho * P:(ho + 1) * P],
                                rhs=rhs_x(nf, hi),
                                start=(hi == 0), stop=(hi == HC - 1))
                        nc.vector.tensor_scalar(
                            out=dstT[ho][:, nf * S:(nf + 1) * S], in0=ps[:],
                            scalar1=bt[:, ho:ho + 1], scalar2=None, op0=OP.add)

                    def v_group(t):
                        pv = psum.tile([P, 2 * S], f32, tag="b", bufs=2, name="psv")
                        for nf, n0, nn in ((0, 0, S), (1, S, H - S)):
                            for hi in range(HC):
                                nc.tensor.matmul(
                                    pv[:, n0:n0 + nn],
                                    lhsT=xT[:, t, hi, :],
                                    rhs=wv_c[hi][:, n0:n0 + nn],
                                    start=(hi == 0), stop=(hi == HC - 1))
                        nc.vector.tensor_copy(
                            Vt[t][:, :, 0:DH],
                            pv[:, 0:H].rearrange("p (h d) -> p h d", h=NH))
                        nc.vector.memset(Vt[t][:, :, DH:DH + 1], 1.0)

                    # --- attention unit pieces (seq s, head hd) ---
                    ex_tiles = {}

                    def sc_unit(s, hd):
                        """scoresT + exp for one (seq, head): 4 kc tiles."""
                        hc, po = hd // 2, (hd % 2) * DH
                        exs = []
                        for kc in range(SC):
                            ps = psum.tile([P, S], f32, tag="a", bufs=2, name="pss")
                            nc.tensor.matmul(
                                ps[:],
                                lhsT=KT[hc][po:po + DH,
                                            s * S + kc * P:s * S + (kc + 1) * P],
                                rhs=QT[hc][po:po + DH, s * S:(s + 1) * S],
                                start=True, stop=True, tile_position=(po, 0))
                            ex = ap_.tile([P, S], bf16, tag="ex", bufs=16, name="ex")
                            if with_mask:
                                nc.scalar.activation(
                                    ex[:], ps[:], AF.Exp,
                                    bias=extm_sb[:, s * SC + kc:s * SC + kc + 1],
                                    scale=INV_SQRT_DH)
                            else:
                                nc.scalar.activation(ex[:], ps[:], AF.Exp,
                                                     scale=INV_SQRT_DH)
                            exs.append(ex)
                        ex_tiles[(s, hd)] = exs

                    def ctx_unit(s, hd):
                        """ctx + denominator + normalize for one (seq, head)."""
                        hc, po = hd // 2, (hd % 2) * DH
                        exs = ex_tiles.pop((s, hd))
                        cx = psum.tile([P, S], f32, tag="c", bufs=2, name="cx")
                        for kc in range(SC):
                            nc.tensor.matmul(
                                cx[0:DH + 1, :],
                                lhsT=Vt[s * SC + kc][:, hd, :],
                                rhs=exs[kc][:],
                                start=(kc == 0), stop=(kc == SC - 1))
                        rinv = small.tile([1, S], f32, tag="rinv", bufs=4)
                        nc.vector.reciprocal(rinv[:], cx[DH:DH + 1, :])
                        rb = small.tile([DH, S], f32, tag="rb", bufs=3)
                        nc.gpsimd.partition_broadcast(rb[:], rinv[:], channels=DH)
                        nc.vector.tensor_tensor(
                            out=ctxT[hc][po:po + DH, s * S:(s + 1) * S],
                            in0=cx[0:DH, :], in1=rb[:], op=OP.mult)

                    def ao_group(t):
                        """attn-out projection + residual + LN1 for chunk t."""
                        po_ = psum.tile([P, 2 * S], f32, tag="b", bufs=2, name="pao")
                        for nf, n0, nn in ((0, 0, S), (1, S, H - S)):
                            for hi in range(HC):
                                nc.tensor.matmul(
                                    po_[:, n0:n0 + nn],
                                    lhsT=ctxT[hi][:, t * P:(t + 1) * P],
                                    rhs=wo_c[hi][:, n0:n0 + nn],
                                    start=(hi == 0),
                                    stop=(hi == HC - 1 and not with_brow))
                            if with_brow:
                                nc.tensor.matmul(po_[:, n0:n0 + nn], lhsT=ones1[:],
                                                 rhs=brow_t[0:1, n0:n0 + nn],
                                                 start=False, stop=True)
                        ln_store(po_[:, 0:H], cur[t][:], nxt[t], t)
                        transpose_t(t)

                    # --- emission schedule ---
                    # Only nf0 (seq0) xT chunks are ready at layer start (the
                    # previous layer's t4..7 LN2 transposes land a few us in),
                    # so everything nf1-dependent is deferred into the seq1
                    # stream.  V(seq0) first so seq0 ctx units can interleave
                    # with the QK/scores stream (ctx lags scores by 2 units:
                    # at most 4 score units = 16 ex tiles in flight).
                    for t in range(SC):
                        v_group(t)
                    for hc in range(HC):
                        qk_group(QT, wq_c, bq_t, hc, 0)
                        qk_group(KT, wk_c, bk_t, hc, 0)
                        if hc > 0:
                            ctx_unit(0, 2 * hc - 2)
                            ctx_unit(0, 2 * hc - 1)
                        sc_unit(0, 2 * hc)
                        sc_unit(0, 2 * hc + 1)
                    # seq1: nf1 QK groups + V(seq1) spread into the scores
                    # pipeline, ctx lags scores by 2
                    ctx_unit(0, NH - 2)
                    ctx_unit(0, NH - 1)
                    for h in range(NH):
                        if h % 2 == 0:
                            qk_group(QT, wq_c, bq_t, h // 2, 1)
                            qk_group(KT, wk_c, bk_t, h // 2, 1)
                        sc_unit(1, h)
                        if h < 2:
                            v_group(SC + 2 * h)
                            v_group(SC + 2 * h + 1)
                        if h >= 2:
                            ctx_unit(1, h - 2)
                    # tail: remaining ctx + seq0 attn-out/LN1 interleaved
                    ctx_unit(1, NH - 2)
                    ao_group(0)
                    ctx_unit(1, NH - 1)
                    ao_group(1)
                    if l == 0:
                        tap("QT", QT)
                        tap("KT", KT)
                        tap("ctxT", ctxT)
                    for t in range(2, TC):
                        ao_group(t)
                    if l == 0:
                        tap("ln1", [nxt[t] for t in range(TC)])

                # --- FFN ---
                with ExitStack() as ffn_scope:
                    fp_ = ffn_scope.enter_context(
                        tc.tile_pool(name=f"ffn{l}", bufs=1))
                    gT = [fp_.tile([P, NT], bf16, name=f"gT{l}_{i}") for i in range(IC)]
                    # W1 split into [128,768] chunks in the rotating pool
                    w1_c = [[wload(dW1, hi * P, j * H) for j in range(IC // HC)]
                            for hi in range(HC)]

                    for i_ in range(IC):
                        for nf in range(2):
                            ps = psum.tile([P, S], f32, tag="a", bufs=2, name="psf")
                            for hi in range(HC):
                                w1t = w1_c[hi][(i_ * P) // H]
                                c0 = (i_ * P) % H
                                nc.tensor.matmul(
                                    ps[:], lhsT=w1t[:, c0:c0 + P],
                                    rhs=rhs_x(nf, hi),
                                    start=(hi == 0), stop=(hi == HC - 1))
                            nc.scalar.activation(
                                gT[i_][:, nf * S:(nf + 1) * S], ps[:], AF.Gelu,
                                bias=b1_t[:, i_:i_ + 1], scale=1.0)
                    if l == 0:
                        tap("gT", gT)

                    w2_c = [wload(dW2, i_ * P, 0) for i_ in range(IC)]
                    for t in range(TC):
                        pf = psum.tile([P, 2 * S], f32, tag="b", bufs=2, name="pf")
                        for i_ in range(IC):
                            for nf, n0, nn in ((0, 0, S), (1, S, H - S)):
                                nc.tensor.matmul(
                                    pf[:, n0:n0 + nn],
                                    lhsT=gT[i_][:, t * P:(t + 1) * P],
                                    rhs=w2_c[i_][:, n0:n0 + nn],
                                    start=(i_ == 0),
                                    stop=(i_ == IC - 1 and not with_brow))
                        if with_brow:
                            for nf, n0, nn in ((0, 0, S), (1, S, H - S)):
                                nc.tensor.matmul(pf[:, n0:n0 + nn], lhsT=ones1[:],
                                                 rhs=brow_t[0:1, H + n0:H + n0 + nn],
                                                 start=False, stop=True)
                        last = (l == layers - 1)
                        ln_store(pf[:, 0:H], nxt[t][:], cur[t], t, last=last,
                                 out_f32=out.ap()[t * P:(t + 1) * P, :])
                        if not last:
                            transpose_t(t)

    nc.compile()
    return nc


# --------------------------------------------------------------------------
# host side
# --------------------------------------------------------------------------

def prep_shared(inputs):
    sh = {}
    sh["wemb_bf"] = inputs["word_emb"].astype(_BF16)
    sh["temb_bf"] = inputs["type_emb"].astype(_BF16)
    sh["pemb"] = inputs["pos_emb"].astype(np.float32)
    for k in ("Wq", "Wk", "Wv", "Wo", "W1", "W2"):
        sh[k] = inputs[k].astype(_BF16)
    sh["bq"] = inputs["bq"].astype(np.float32)
    sh["bk"] = inputs["bk"].astype(np.float32)
    sh["b1"] = inputs["b1"].astype(np.float32)
    # brow rows: [bv @ Wo + bo, b2]  (bv folded through the out-projection)
    bvWo = np.einsum("lh,lho->lo", inputs["bv"].astype(np.float64),
                     inputs["Wo"].astype(np.float64)).astype(np.float32)
    browo = (bvWo + inputs["bo"]).astype(np.float32)
    sh["brow"] = np.concatenate([browo, inputs["b2"]],
                                axis=1)[:, None, :].astype(_BF16)
    return sh


def core_inputs(inputs, sh, c):
    ids = np.asarray(inputs["input_ids"]).astype(np.int64)
    tts = np.asarray(inputs["token_type_ids"]).astype(np.int64)
    am = np.asarray(inputs["attention_mask"]).astype(np.float32)
    b0 = c * B_LOC
    m = {k: v for k, v in sh.items() if k not in ("wemb_bf", "temb_bf")}
    m["wrows"] = np.ascontiguousarray(sh["wemb_bf"][ids[b0:b0 + B_LOC].reshape(-1)])
    m["trows"] = np.ascontiguousarray(sh["temb_bf"][tts[b0:b0 + B_LOC].reshape(-1)])
    # [P, B_LOC*SC] per-partition mask columns
    em = ((1.0 - am[b0:b0 + B_LOC]) * -10000.0).reshape(B_LOC * SC, P)
    m["extmc"] = np.ascontiguousarray(em.T).astype(np.float32)
    return m


_NC_CACHE = {}


def flags_for(inputs):
    with_mask = not np.all(np.asarray(inputs["attention_mask"]) == 1.0)
    with_brow = bool(np.any(np.asarray(inputs["bo"])) or
                     np.any(np.asarray(inputs["b2"])) or
                     np.any(np.asarray(inputs["bv"])))
    return with_mask, with_brow


def get_nc(layers=L, with_mask=False, with_brow=False):
    key = (layers, with_mask, with_brow)
    if key not in _NC_CACHE:
        _NC_CACHE[key] = build(layers, with_mask=with_mask, with_brow=with_brow)
    return _NC_CACHE[key]


def run(inputs, layers=L):
    from concourse.bass_utils import run_bass_kernel_spmd
    inputs = {k: np.asarray(v) for k, v in inputs.items()}
    wm, wb = flags_for(inputs)
    nc = get_nc(layers, wm, wb)
    sh = prep_shared(inputs)
    in_maps = [core_inputs(inputs, sh, c) for c in range(NCORES)]
    res = run_bass_kernel_spmd(nc, in_maps, core_ids=list(range(NCORES)))
    outs = [res.results[c]["out"].reshape(B_LOC, S, H) for c in range(NCORES)]
    return np.concatenate(outs, axis=0).astype(np.float32)


def kernel(**inputs):
    return run(inputs)


# revision 13
# speedup vs baseline: 1.0424x; 1.0424x over previous
"""BERT-base forward on 8 Trainium2 NeuronCores.

Strategy: pure data parallelism over the batch (B=16 -> 2 sequences per
core), weights replicated, zero collectives.

v6 redesign (from v5's trace: ACT 52%/2.4ms + DVE 36%/1.66ms serialized
against PE 85%/3.9ms, 29% of the span HAM-throttled at half clock):

* k-major attention: scores are computed transposed ([k, q]) so the
  softmax probabilities feed the ctx matmul directly -- the 384 PE
  transpose-matmuls + 96 psum copies per layer of v5 are gone.  The
  softmax denominator comes for free as row 64 of the ctx matmul
  (V carries an appended ones column, M=65); normalization happens
  after ctx via DVE-recip -> gpsimd partition-broadcast -> DVE mult.
* token->feature layout changes (xtok -> xT) run on the DMA xbar
  transpose engine instead of PE matmul + ACT copy.
* PSUM evacuations moved from ACT to DVE; ACT keeps exp/gelu/sqrt and
  the bf16 xtok copies only.
* 1/sqrt(DH) folded into the exp scale; bv folded into the attn-out
  row bias host-side (brow_o = bv @ Wo + bo); mask folded into the
  exp per-partition bias (k-major makes the mask per-partition).
* single rotating weight-chunk pool ([128,768] bf16 x 24 bufs) streams
  Wq/Wk/Wv/Wo/W1/W2 and prefetches across phases.

Layout conventions per core (P=128 partitions):
  tokens NT=1024 (2 seqs x 512), token chunk t in [0,8)
  token-major  [128 tokens, H]   - residual stream, layernorm
  feature-major xT [128, t, hc, 128] - matmul lhsT/rhs operands
  matmul computes out = lhsT.T @ rhs (contraction along partitions)
"""

import numpy as np
import ml_dtypes

V, H, L, NH, I, S = 30522, 768, 12, 12, 3072, 512
B_FULL, NCORES, B_LOC = 16, 8, 2
DH = H // NH                      # 64
P = 128
NT = B_LOC * S                    # 1024 tokens per core
TC = NT // P                      # 8 token chunks
HC = H // P                       # 6 feature chunks
IC = I // P                       # 24 ffn chunks
SC = S // P                       # 4 chunks per sequence
EPS = 1e-12
INV_SQRT_DH = 1.0 / 8.0

_BF16 = ml_dtypes.bfloat16


# --------------------------------------------------------------------------
# device kernel builder
# --------------------------------------------------------------------------

def build(layers=L, taps=None, with_mask=False, with_brow=False):
    import concourse.bass as bass
    import concourse.mybir as mybir
    import concourse.tile as tile
    from concourse import bacc
    from contextlib import ExitStack

    dt = mybir.dt
    AF = mybir.ActivationFunctionType
    OP = mybir.AluOpType

    nc = bacc.Bacc("TRN2", target_bir_lowering=False, debug=False,
                   num_devices=NCORES)

    # ---- DRAM inputs (per core) ----
    wrows = nc.dram_tensor("wrows", [NT, H], dt.bfloat16, kind="ExternalInput")
    trows = nc.dram_tensor("trows", [NT, H], dt.bfloat16, kind="ExternalInput")
    pemb = nc.dram_tensor("pemb", [S, H], dt.float32, kind="ExternalInput")
    # mask as per-partition columns: extmc[p, s*SC+kc] = mask bias of
    # k-token s*512 + kc*128 + p (k-major scores make the mask per-partition)
    extmc = nc.dram_tensor("extmc", [P, B_LOC * SC], dt.float32,
                           kind="ExternalInput")
    dWq = nc.dram_tensor("Wq", [L, H, H], dt.bfloat16, kind="ExternalInput")
    dWk = nc.dram_tensor("Wk", [L, H, H], dt.bfloat16, kind="ExternalInput")
    dWv = nc.dram_tensor("Wv", [L, H, H], dt.bfloat16, kind="ExternalInput")
    dWo = nc.dram_tensor("Wo", [L, H, H], dt.bfloat16, kind="ExternalInput")
    dW1 = nc.dram_tensor("W1", [L, H, I], dt.bfloat16, kind="ExternalInput")
    dW2 = nc.dram_tensor("W2", [L, I, H], dt.bfloat16, kind="ExternalInput")
    dbq = nc.dram_tensor("bq", [L, H], dt.float32, kind="ExternalInput")
    dbk = nc.dram_tensor("bk", [L, H], dt.float32, kind="ExternalInput")
    db1 = nc.dram_tensor("b1", [L, I], dt.float32, kind="ExternalInput")
    # free-dim biases (added via K=1 rank-1 matmuls): rows [bv@Wo+bo, b2]
    dbrow = nc.dram_tensor("brow", [L, 1, 2 * H], dt.bfloat16, kind="ExternalInput")
    out = nc.dram_tensor("out", [NT, H], dt.float32, kind="ExternalOutput")

    f32, bf16 = dt.float32, dt.bfloat16

    def tap(name, tiles):
        if taps is None:
            return
        sh0 = list(tiles[0].shape)
        d = nc.dram_tensor(f"tap_{name}", [len(tiles)] + sh0,
                           tiles[0].dtype, kind="ExternalOutput")
        for i, t in enumerate(tiles):
            nc.sync.dma_start(d.ap()[i], t[:])
        taps[name] = d

    with tile.TileContext(nc) as tc_, ExitStack() as top:
        tc = tc_

        # ---- constants & persistent activation tiles ----
        pers = top.enter_context(tc.tile_pool(name="pers", bufs=1))
        ones1 = pers.tile([1, P], bf16, name="ones1")
        nc.vector.memset(ones1[:], 1.0)
        eps_t = pers.tile([P, 1], f32, name="eps_t")
        nc.vector.memset(eps_t[:], EPS)
        extm_sb = pers.tile([P, B_LOC * SC], f32, name="extm_sb")
        nc.sync.dma_start(extm_sb[:], extmc.ap())

        curA = [pers.tile([P, H], f32, name=f"curA{t}") for t in range(TC)]
        curB = [pers.tile([P, H], f32, name=f"curB{t}") for t in range(TC)]
        xtok = [pers.tile([P, H], bf16, name=f"xtok{t}") for t in range(TC)]
        # feature-major activations: xT[p, t, hc, j] = x[token t*128+j,
        # feature hc*128+p], written by DMA xbar transpose per t-chunk
        xT = pers.tile([P, TC, HC, P], bf16, name="xT")

        small = top.enter_context(tc.tile_pool(name="small", bufs=6))
        psum = top.enter_context(tc.tile_pool(name="psum", space="PSUM", bufs=1))
        # psum tags: "a" scores/QK/FFN1 [128,512]x2; "c" ctx [128,512]x2;
        # "b" V/attn-out/FFN2 [128,1024]x2 (768 used).  2+2+4 banks = 8.

        def rhs_x(nf, hi):
            """feature-major rhs [128, 512] for sequence-half nf, chunk hi."""
            return xT[:, 4 * nf:4 * nf + 4, hi, :]

        # ---------------- helpers ----------------
        def ln_store(src_ap, res_ap, dst, tcid, last=False, out_f32=None):
            """dst = layernorm(src + res); also writes bf16 copy to xtok[tcid]
            unless last (then DMAs fp32 to out_f32)."""
            s1 = small.tile([P, 1], f32, tag="s1")
            nc.vector.scalar_tensor_tensor(
                out=dst[:], in0=src_ap, scalar=0.0, in1=res_ap,
                op0=OP.add, op1=OP.add, accum_out=s1[:])
            u = small.tile([P, 1], f32, tag="u")
            nc.vector.tensor_scalar(out=u[:], in0=s1[:], scalar1=1.0 / H,
                                    scalar2=None, op0=OP.mult)
            junk = small.tile([P, H], f32, tag="junk", bufs=2)
            s2 = small.tile([P, 1], f32, tag="s2")
            nc.vector.scalar_tensor_tensor(
                out=junk[:], in0=dst[:], scalar=u[:], in1=dst[:],
                op0=OP.subtract, op1=OP.mult, accum_out=s2[:])
            sd = small.tile([P, 1], f32, tag="sd")
            nc.scalar.activation(sd[:], s2[:], AF.Sqrt, bias=eps_t[:], scale=1.0 / H)
            rstd = small.tile([P, 1], f32, tag="rstd")
            nc.vector.reciprocal(rstd[:], sd[:])
            nc.vector.tensor_scalar(out=dst[:], in0=dst[:], scalar1=u[:],
                                    scalar2=rstd[:], op0=OP.subtract, op1=OP.mult)
            if last:
                nc.scalar.dma_start(out_f32, dst[:])
            else:
                nc.scalar.copy(xtok[tcid][:], dst[:])

        def transpose_t(t):
            """xtok[t] (token-major bf16) -> xT[:, t] via DMA xbar.

            On the sync HWDGE queue; weight loads go through the gpsimd
            SWDGE queue so a transpose waiting on its xtok copy never
            head-of-line-blocks weight prefetch (queues are FIFO)."""
            nc.sync.dma_start_transpose(xT[:, t], xtok[t][:])

        # ---- embedding: gather + add + LN ----
        with ExitStack() as emb_scope:
            ep = emb_scope.enter_context(tc.tile_pool(name="emb", bufs=1))
            wg = ep.tile([P, TC, H], bf16, name="wg")
            tg = ep.tile([P, TC, H], bf16, name="tg")
            nc.sync.dma_start(wg[:], wrows.ap().rearrange("(c p) h -> p c h", p=P))
            nc.sync.dma_start(tg[:], trows.ap().rearrange("(c p) h -> p c h", p=P))
            pos = ep.tile([P, SC, H], f32, name="pos")
            nc.sync.dma_start(pos[:], pemb.ap().rearrange("(c p) h -> p c h", p=P))
            for t in range(TC):
                tmp = ep.tile([P, H], f32, tag="etmp", bufs=2, name="etmp")
                nc.vector.tensor_tensor(out=tmp[:], in0=tg[:, t],
                                        in1=pos[:, t % SC], op=OP.add)
                ln_store(wg[:, t], tmp[:], curA[t], t)
                transpose_t(t)
            tap("emb", curA)

        # ---- transformer layers ----
        for l in range(layers):
            with ExitStack() as ls:
                wp = ls.enter_context(tc.tile_pool(name=f"bias{l}", bufs=1))
                bq_t = wp.tile([P, HC], f32, name=f"bq{l}")
                bk_t = wp.tile([P, HC], f32, name=f"bk{l}")
                b1_t = wp.tile([P, IC], f32, name=f"b1{l}")
                nc.sync.dma_start(bq_t[:], dbq.ap()[l].rearrange("(c p) -> p c", p=P))
                nc.sync.dma_start(bk_t[:], dbk.ap()[l].rearrange("(c p) -> p c", p=P))
                nc.sync.dma_start(b1_t[:], db1.ap()[l].rearrange("(c p) -> p c", p=P))
                brow_t = wp.tile([1, 2 * H], bf16, name=f"brow{l}")
                nc.sync.dma_start(brow_t[:], dbrow.ap()[l])

                # single rotating pool for all weight chunks of this layer
                wpool = ls.enter_context(tc.tile_pool(name=f"w{l}", bufs=24))

                def wload(dW, r0, c0, cols=H):
                    w = wpool.tile([P, H], bf16, tag="wc")
                    nc.gpsimd.dma_start(w[:, :cols],
                                        dW.ap()[l, r0:r0 + P, c0:c0 + cols])
                    return w

                cur, nxt = (curA, curB)

                with ExitStack() as attn_scope:
                    ap_ = attn_scope.enter_context(
                        tc.tile_pool(name=f"attn{l}", bufs=1))
                    QT = [ap_.tile([P, NT], bf16, name=f"QT{l}_{h}") for h in range(HC)]
                    KT = [ap_.tile([P, NT], bf16, name=f"KT{l}_{h}") for h in range(HC)]
                    # V with an appended ones column per head (65 cols/head)
                    Vt = [ap_.tile([P, NH, DH + 1], bf16, name=f"V{l}_{t}")
                          for t in range(TC)]
                    ctxT = [ap_.tile([P, NT], bf16, name=f"cT{l}_{h}") for h in range(HC)]

                    wv_c = [wload(dWv, h * P, 0) for h in range(HC)]
                    wq_c = [wload(dWq, h * P, 0) for h in range(HC)]
                    wk_c = [wload(dWk, h * P, 0) for h in range(HC)]
                    wo_c = [wload(dWo, h * P, 0) for h in range(HC)]

                    def qk_group(dstT, wch, bt, ho, nf):
                        ps = psum.tile([P, S], f32, tag="a", bufs=2, name="psqk")
                        for hi in range(HC):
                            nc.tensor.matmul(
                                ps[:], lhsT=wch[hi][:, ho * P:(ho + 1) * P],
                                rhs=rhs_x(nf, hi),
                                start=(hi == 0), stop=(hi == HC - 1))
                        nc.vector.tensor_scalar(
                            out=dstT[ho][:, nf * S:(nf + 1) * S], in0=ps[:],
                            scalar1=bt[:, ho:ho + 1], scalar2=None, op0=OP.add)

                    def v_group(t):
                        pv = psum.tile([P, 2 * S], f32, tag="b", bufs=2, name="psv")
                        for nf, n0, nn in ((0, 0, S), (1, S, H - S)):
                            for hi in range(HC):
                                nc.tensor.matmul(
                                    pv[:, n0:n0 + nn],
                                    lhsT=xT[:, t, hi, :],
                                    rhs=wv_c[hi][:, n0:n0 + nn],
                                    start=(hi == 0), stop=(hi == HC - 1))
                        nc.vector.tensor_copy(
                            Vt[t][:, :, 0:DH],
                            pv[:, 0:H].rearrange("p (h d) -> p h d", h=NH))
                        nc.vector.memset(Vt[t][:, :, DH:DH + 1], 1.0)

                    # --- attention unit pieces (seq s, head hd) ---
                    ex_tiles = {}

                    def sc_unit(s, hd):
                        """scoresT + exp for one (seq, head): 4 kc tiles."""
                        hc, po = hd // 2, (hd % 2) * DH
                        exs = []
                        for kc in range(SC):
                            ps = psum.tile([P, S], f32, tag="a", bufs=2, name="pss")
                            nc.tensor.matmul(
                                ps[:],
                                lhsT=KT[hc][po:po + DH,
                                            s * S + kc * P:s * S + (kc + 1) * P],
                                rhs=QT[hc][po:po + DH, s * S:(s + 1) * S],
                                start=True, stop=True, tile_position=(po, 0))
                            ex = ap_.tile([P, S], bf16, tag="ex", bufs=16, name="ex")
                            if with_mask:
                                nc.scalar.activation(
                                    ex[:], ps[:], AF.Exp,
                                    bias=extm_sb[:, s * SC + kc:s * SC + kc + 1],
                                    scale=INV_SQRT_DH)
                            else:
                                nc.scalar.activation(ex[:], ps[:], AF.Exp,
                                                     scale=INV_SQRT_DH)
                            exs.append(ex)
                        ex_tiles[(s, hd)] = exs

                    def ctx_unit(s, hd):
                        """ctx + denominator + normalize for one (seq, head)."""
                        hc, po = hd // 2, (hd % 2) * DH
                        exs = ex_tiles.pop((s, hd))
                        cx = psum.tile([P, S], f32, tag="c", bufs=2, name="cx")
                        for kc in range(SC):
                            nc.tensor.matmul(
                                cx[0:DH + 1, :],
                                lhsT=Vt[s * SC + kc][:, hd, :],
                                rhs=exs[kc][:],
                                start=(kc == 0), stop=(kc == SC - 1))
                        rinv = small.tile([1, S], f32, tag="rinv", bufs=4)
                        nc.vector.reciprocal(rinv[:], cx[DH:DH + 1, :])
                        rb = small.tile([DH, S], f32, tag="rb", bufs=3)
                        nc.gpsimd.partition_broadcast(rb[:], rinv[:], channels=DH)
                        nc.vector.tensor_tensor(
                            out=ctxT[hc][po:po + DH, s * S:(s + 1) * S],
                            in0=cx[0:DH, :], in1=rb[:], op=OP.mult)

                    def ao_group(t):
                        """attn-out projection + residual + LN1 for chunk t."""
                        po_ = psum.tile([P, 2 * S], f32, tag="b", bufs=2, name="pao")
                        for nf, n0, nn in ((0, 0, S), (1, S, H - S)):
                            for hi in range(HC):
                                nc.tensor.matmul(
                                    po_[:, n0:n0 + nn],
                                    lhsT=ctxT[hi][:, t * P:(t + 1) * P],
                                    rhs=wo_c[hi][:, n0:n0 + nn],
                                    start=(hi == 0),
                                    stop=(hi == HC - 1 and not with_brow))
                            if with_brow:
                                nc.tensor.matmul(po_[:, n0:n0 + nn], lhsT=ones1[:],
                                                 rhs=brow_t[0:1, n0:n0 + nn],
                                                 start=False, stop=True)
                        ln_store(po_[:, 0:H], cur[t][:], nxt[t], t)
                        transpose_t(t)

                    # --- emission schedule ---
                    # Only nf0 (seq0) xT chunks are ready at layer start (the
                    # previous layer's t4..7 LN2 transposes land a few us in),
                    # so everything nf1-dependent is deferred into the seq1
                    # stream.  V(seq0) first so seq0 ctx units can interleave
                    # with the QK/scores stream (ctx lags scores by 2 units:
                    # at most 4 score units = 16 ex tiles in flight).
                    for t in range(SC):
                        v_group(t)
                    for hc in range(HC):
                        qk_group(QT, wq_c, bq_t, hc, 0)
                        qk_group(KT, wk_c, bk_t, hc, 0)
                        if hc > 0:
                            ctx_unit(0, 2 * hc - 2)
                            ctx_unit(0, 2 * hc - 1)
                        sc_unit(0, 2 * hc)
                        sc_unit(0, 2 * hc + 1)
                    # seq1: nf1 QK groups + V(seq1) spread into the scores
                    # pipeline, ctx lags scores by 2
                    ctx_unit(0, NH - 2)
                    ctx_unit(0, NH - 1)
                    for h in range(NH):
                        if h % 2 == 0:
                            qk_group(QT, wq_c, bq_t, h // 2, 1)
                            qk_group(KT, wk_c, bk_t, h // 2, 1)
                        sc_unit(1, h)
                        if h < 2:
                            v_group(SC + 2 * h)
                            v_group(SC + 2 * h + 1)
                        if h >= 2:
                            ctx_unit(1, h - 2)
                        if h >= NH - SC:
                            ao_group(h - (NH - SC))  # t0..3 (seq0 ctx done)
                    # tail: remaining ctx + seq1 attn-out/LN1
                    ctx_unit(1, NH - 2)
                    ctx_unit(1, NH - 1)
                    if l == 0:
                        tap("QT", QT)
                        tap("KT", KT)
                        tap("ctxT", ctxT)
                    for t in range(SC, TC):
                        ao_group(t)
                    if l == 0:
                        tap("ln1", [nxt[t] for t in range(TC)])

                # --- FFN ---
                with ExitStack() as ffn_scope:
                    fp_ = ffn_scope.enter_context(
                        tc.tile_pool(name=f"ffn{l}", bufs=1))
                    gT = [fp_.tile([P, NT], bf16, name=f"gT{l}_{i}") for i in range(IC)]
                    # W1 split into [128,768] chunks in the rotating pool
                    w1_c = [[wload(dW1, hi * P, j * H) for j in range(IC // HC)]
                            for hi in range(HC)]

                    # nf-outer: the 24 nf0 groups cover the latency of the
                    # seq1 LN1 chains + transposes that nf1 depends on
                    for nf in range(2):
                        for i_ in range(IC):
                            ps = psum.tile([P, S], f32, tag="a", bufs=2, name="psf")
                            for hi in range(HC):
                                w1t = w1_c[hi][(i_ * P) // H]
                                c0 = (i_ * P) % H
                                nc.tensor.matmul(
                                    ps[:], lhsT=w1t[:, c0:c0 + P],
                                    rhs=rhs_x(nf, hi),
                                    start=(hi == 0), stop=(hi == HC - 1))
                            nc.scalar.activation(
                                gT[i_][:, nf * S:(nf + 1) * S], ps[:], AF.Gelu,
                                bias=b1_t[:, i_:i_ + 1], scale=1.0)
                    if l == 0:
                        tap("gT", gT)

                    w2_c = [wload(dW2, i_ * P, 0) for i_ in range(IC)]
                    for t in range(TC):
                        pf = psum.tile([P, 2 * S], f32, tag="b", bufs=2, name="pf")
                        for i_ in range(IC):
                            for nf, n0, nn in ((0, 0, S), (1, S, H - S)):
                                nc.tensor.matmul(
                                    pf[:, n0:n0 + nn],
                                    lhsT=gT[i_][:, t * P:(t + 1) * P],
                                    rhs=w2_c[i_][:, n0:n0 + nn],
                                    start=(i_ == 0),
                                    stop=(i_ == IC - 1 and not with_brow))
                        if with_brow:
                            for nf, n0, nn in ((0, 0, S), (1, S, H - S)):
                                nc.tensor.matmul(pf[:, n0:n0 + nn], lhsT=ones1[:],
                                                 rhs=brow_t[0:1, H + n0:H + n0 + nn],
                                                 start=False, stop=True)
                        last = (l == layers - 1)
                        ln_store(pf[:, 0:H], nxt[t][:], cur[t], t, last=last,
                                 out_f32=out.ap()[t * P:(t + 1) * P, :])
                        if not last:
                            transpose_t(t)

    nc.compile()
    return nc


# --------------------------------------------------------------------------
# host side
# --------------------------------------------------------------------------

def prep_shared(inputs):
    sh = {}
    sh["wemb_bf"] = inputs["word_emb"].astype(_BF16)
    sh["temb_bf"] = inputs["type_emb"].astype(_BF16)
    sh["pemb"] = inputs["pos_emb"].astype(np.float32)
    for k in ("Wq", "Wk", "Wv", "Wo", "W1", "W2"):
        sh[k] = inputs[k].astype(_BF16)
    sh["bq"] = inputs["bq"].astype(np.float32)
    sh["bk"] = inputs["bk"].astype(np.float32)
    sh["b1"] = inputs["b1"].astype(np.float32)
    # brow rows: [bv @ Wo + bo, b2]  (bv folded through the out-projection)
    bvWo = np.einsum("lh,lho->lo", inputs["bv"].astype(np.float64),
                     inputs["Wo"].astype(np.float64)).astype(np.float32)
    browo = (bvWo + inputs["bo"]).astype(np.float32)
    sh["brow"] = np.concatenate([browo, inputs["b2"]],
                                axis=1)[:, None, :].astype(_BF16)
    return sh


def core_inputs(inputs, sh, c):
    ids = np.asarray(inputs["input_ids"]).astype(np.int64)
    tts = np.asarray(inputs["token_type_ids"]).astype(np.int64)
    am = np.asarray(inputs["attention_mask"]).astype(np.float32)
    b0 = c * B_LOC
    m = {k: v for k, v in sh.items() if k not in ("wemb_bf", "temb_bf")}
    m["wrows"] = np.ascontiguousarray(sh["wemb_bf"][ids[b0:b0 + B_LOC].reshape(-1)])
    m["trows"] = np.ascontiguousarray(sh["temb_bf"][tts[b0:b0 + B_LOC].reshape(-1)])
    # [P, B_LOC*SC] per-partition mask columns
    em = ((1.0 - am[b0:b0 + B_LOC]) * -10000.0).reshape(B_LOC * SC, P)
    m["extmc"] = np.ascontiguousarray(em.T).astype(np.float32)
    return m


_NC_CACHE = {}


def flags_for(inputs):
    with_mask = not np.all(np.asarray(inputs["attention_mask"]) == 1.0)
    with_brow = bool(np.any(np.asarray(inputs["bo"])) or
                     np.any(np.asarray(inputs["b2"])) or
                     np.any(np.asarray(inputs["bv"])))
    return with_mask, with_brow


def get_nc(layers=L, with_mask=False, with_brow=False):
    key = (layers, with_mask, with_brow)
    if key not in _NC_CACHE:
        _NC_CACHE[key] = build(layers, with_mask=with_mask, with_brow=with_brow)
    return _NC_CACHE[key]


def run(inputs, layers=L):
    from concourse.bass_utils import run_bass_kernel_spmd
    inputs = {k: np.asarray(v) for k, v in inputs.items()}
    wm, wb = flags_for(inputs)
    nc = get_nc(layers, wm, wb)
    sh = prep_shared(inputs)
    in_maps = [core_inputs(inputs, sh, c) for c in range(NCORES)]
    res = run_bass_kernel_spmd(nc, in_maps, core_ids=list(range(NCORES)))
    outs = [res.results[c]["out"].reshape(B_LOC, S, H) for c in range(NCORES)]
    return np.concatenate(outs, axis=0).astype(np.float32)


def kernel(**inputs):
    return run(inputs)


# revision 15
# speedup vs baseline: 1.1391x; 1.0927x over previous
"""BERT-base forward on 8 Trainium2 NeuronCores.

Strategy: pure data parallelism over the batch (B=16 -> 2 sequences per
core), weights replicated, zero collectives.

v6 redesign (from v5's trace: ACT 52%/2.4ms + DVE 36%/1.66ms serialized
against PE 85%/3.9ms, 29% of the span HAM-throttled at half clock):

* k-major attention: scores are computed transposed ([k, q]) so the
  softmax probabilities feed the ctx matmul directly -- the 384 PE
  transpose-matmuls + 96 psum copies per layer of v5 are gone.  The
  softmax denominator comes for free as row 64 of the ctx matmul
  (V carries an appended ones column, M=65); normalization happens
  after ctx via DVE-recip -> gpsimd partition-broadcast -> DVE mult.
* token->feature layout changes (xtok -> xT) run on the DMA xbar
  transpose engine instead of PE matmul + ACT copy.
* PSUM evacuations moved from ACT to DVE; ACT keeps exp/gelu/sqrt and
  the bf16 xtok copies only.
* 1/sqrt(DH) folded into the exp scale; bv folded into the attn-out
  row bias host-side (brow_o = bv @ Wo + bo); mask folded into the
  exp per-partition bias (k-major makes the mask per-partition).
* single rotating weight-chunk pool ([128,768] bf16 x 24 bufs) streams
  Wq/Wk/Wv/Wo/W1/W2 and prefetches across phases.

Layout conventions per core (P=128 partitions):
  tokens NT=1024 (2 seqs x 512), token chunk t in [0,8)
  token-major  [128 tokens, H]   - residual stream, layernorm
  feature-major xT [128, t, hc, 128] - matmul lhsT/rhs operands
  matmul computes out = lhsT.T @ rhs (contraction along partitions)
"""

import numpy as np
import ml_dtypes

V, H, L, NH, I, S = 30522, 768, 12, 12, 3072, 512
B_FULL, NCORES, B_LOC = 16, 8, 2
DH = H // NH                      # 64
P = 128
NT = B_LOC * S                    # 1024 tokens per core
TC = NT // P                      # 8 token chunks
HC = H // P                       # 6 feature chunks
IC = I // P                       # 24 ffn chunks
SC = S // P                       # 4 chunks per sequence
EPS = 1e-12
INV_SQRT_DH = 1.0 / 8.0

_BF16 = ml_dtypes.bfloat16


# --------------------------------------------------------------------------
# device kernel builder
# --------------------------------------------------------------------------

def build(layers=L, taps=None, with_mask=False, with_brow=False):
    import concourse.bass as bass
    import concourse.mybir as mybir
    import concourse.tile as tile
    from concourse import bacc
    from contextlib import ExitStack

    dt = mybir.dt
    AF = mybir.ActivationFunctionType
    OP = mybir.AluOpType

    nc = bacc.Bacc("TRN2", target_bir_lowering=False, debug=False,
                   num_devices=NCORES)

    # ---- DRAM inputs (per core) ----
    wrows = nc.dram_tensor("wrows", [NT, H], dt.bfloat16, kind="ExternalInput")
    trows = nc.dram_tensor("trows", [NT, H], dt.bfloat16, kind="ExternalInput")
    pemb = nc.dram_tensor("pemb", [S, H], dt.float32, kind="ExternalInput")
    # mask as per-partition columns: extmc[p, s*SC+kc] = mask bias of
    # k-token s*512 + kc*128 + p (k-major scores make the mask per-partition)
    extmc = nc.dram_tensor("extmc", [P, B_LOC * SC], dt.float32,
                           kind="ExternalInput")
    dWq = nc.dram_tensor("Wq", [L, H, H], dt.bfloat16, kind="ExternalInput")
    dWk = nc.dram_tensor("Wk", [L, H, H], dt.bfloat16, kind="ExternalInput")
    dWv = nc.dram_tensor("Wv", [L, H, H], dt.bfloat16, kind="ExternalInput")
    dWo = nc.dram_tensor("Wo", [L, H, H], dt.bfloat16, kind="ExternalInput")
    dW1 = nc.dram_tensor("W1", [L, H, I], dt.bfloat16, kind="ExternalInput")
    dW2 = nc.dram_tensor("W2", [L, I, H], dt.bfloat16, kind="ExternalInput")
    dbq = nc.dram_tensor("bq", [L, H], dt.float32, kind="ExternalInput")
    dbk = nc.dram_tensor("bk", [L, H], dt.float32, kind="ExternalInput")
    db1 = nc.dram_tensor("b1", [L, I], dt.float32, kind="ExternalInput")
    # free-dim biases (added via K=1 rank-1 matmuls): rows [bv@Wo+bo, b2]
    dbrow = nc.dram_tensor("brow", [L, 1, 2 * H], dt.bfloat16, kind="ExternalInput")
    out = nc.dram_tensor("out", [NT, H], dt.float32, kind="ExternalOutput")

    f32, bf16 = dt.float32, dt.bfloat16

    def tap(name, tiles):
        if taps is None:
            return
        sh0 = list(tiles[0].shape)
        d = nc.dram_tensor(f"tap_{name}", [len(tiles)] + sh0,
                           tiles[0].dtype, kind="ExternalOutput")
        for i, t in enumerate(tiles):
            nc.sync.dma_start(d.ap()[i], t[:])
        taps[name] = d

    with tile.TileContext(nc) as tc_, ExitStack() as top:
        tc = tc_

        # ---- constants & persistent activation tiles ----
        pers = top.enter_context(tc.tile_pool(name="pers", bufs=1))
        ones1 = pers.tile([1, P], bf16, name="ones1")
        nc.vector.memset(ones1[:], 1.0)
        eps_t = pers.tile([P, 1], f32, name="eps_t")
        nc.vector.memset(eps_t[:], EPS)
        extm_sb = pers.tile([P, B_LOC * SC], f32, name="extm_sb")
        nc.sync.dma_start(extm_sb[:], extmc.ap())

        curA = [pers.tile([P, H], f32, name=f"curA{t}") for t in range(TC)]
        curB = [pers.tile([P, H], f32, name=f"curB{t}") for t in range(TC)]
        xtok = [pers.tile([P, H], bf16, name=f"xtok{t}") for t in range(TC)]
        # feature-major activations: xT[p, t, hc, j] = x[token t*128+j,
        # feature hc*128+p], written by DMA xbar transpose per t-chunk
        xT = pers.tile([P, TC, HC, P], bf16, name="xT")

        small = top.enter_context(tc.tile_pool(name="small", bufs=6))
        psum = top.enter_context(tc.tile_pool(name="psum", space="PSUM", bufs=1))
        # psum tags: "a" scores/QK/FFN1 [128,512]x2; "c" ctx [128,512]x2;
        # "b" V/attn-out/FFN2 [128,1024]x2 (768 used).  2+2+4 banks = 8.

        def rhs_x(nf, hi):
            """feature-major rhs [128, 512] for sequence-half nf, chunk hi."""
            return xT[:, 4 * nf:4 * nf + 4, hi, :]

        # ---------------- helpers ----------------
        def ln_store(src_ap, res_ap, dst, tcid, last=False, out_f32=None):
            """dst = layernorm(src + res); also writes bf16 copy to xtok[tcid]
            unless last (then DMAs fp32 to out_f32)."""
            s1 = small.tile([P, 1], f32, tag="s1")
            nc.vector.scalar_tensor_tensor(
                out=dst[:], in0=src_ap, scalar=0.0, in1=res_ap,
                op0=OP.add, op1=OP.add, accum_out=s1[:])
            u = small.tile([P, 1], f32, tag="u")
            nc.vector.tensor_scalar(out=u[:], in0=s1[:], scalar1=1.0 / H,
                                    scalar2=None, op0=OP.mult)
            junk = small.tile([P, H], f32, tag="junk", bufs=2)
            s2 = small.tile([P, 1], f32, tag="s2")
            nc.vector.scalar_tensor_tensor(
                out=junk[:], in0=dst[:], scalar=u[:], in1=dst[:],
                op0=OP.subtract, op1=OP.mult, accum_out=s2[:])
            sd = small.tile([P, 1], f32, tag="sd")
            nc.scalar.activation(sd[:], s2[:], AF.Sqrt, bias=eps_t[:], scale=1.0 / H)
            rstd = small.tile([P, 1], f32, tag="rstd")
            nc.vector.reciprocal(rstd[:], sd[:])
            nc.vector.tensor_scalar(out=dst[:], in0=dst[:], scalar1=u[:],
                                    scalar2=rstd[:], op0=OP.subtract, op1=OP.mult)
            if last:
                nc.scalar.dma_start(out_f32, dst[:])
            else:
                nc.scalar.copy(xtok[tcid][:], dst[:])

        def transpose_t(t):
            """xtok[t] (token-major bf16) -> xT[:, t] via DMA xbar.

            On the sync HWDGE queue; weight loads go through the gpsimd
            SWDGE queue so a transpose waiting on its xtok copy never
            head-of-line-blocks weight prefetch (queues are FIFO)."""
            nc.sync.dma_start_transpose(xT[:, t], xtok[t][:])

        # ---- embedding: gather + add + LN ----
        with ExitStack() as emb_scope:
            ep = emb_scope.enter_context(tc.tile_pool(name="emb", bufs=1))
            wg = ep.tile([P, TC, H], bf16, name="wg")
            tg = ep.tile([P, TC, H], bf16, name="tg")
            nc.sync.dma_start(wg[:], wrows.ap().rearrange("(c p) h -> p c h", p=P))
            nc.sync.dma_start(tg[:], trows.ap().rearrange("(c p) h -> p c h", p=P))
            pos = ep.tile([P, SC, H], f32, name="pos")
            nc.sync.dma_start(pos[:], pemb.ap().rearrange("(c p) h -> p c h", p=P))
            for t in range(TC):
                tmp = ep.tile([P, H], f32, tag="etmp", bufs=2, name="etmp")
                nc.vector.tensor_tensor(out=tmp[:], in0=tg[:, t],
                                        in1=pos[:, t % SC], op=OP.add)
                ln_store(wg[:, t], tmp[:], curA[t], t)
                transpose_t(t)
            tap("emb", curA)

        # ---- transformer layers ----
        for l in range(layers):
            with ExitStack() as ls:
                wp = ls.enter_context(tc.tile_pool(name=f"bias{l}", bufs=1))
                bq_t = wp.tile([P, HC], f32, name=f"bq{l}")
                bk_t = wp.tile([P, HC], f32, name=f"bk{l}")
                b1_t = wp.tile([P, IC], f32, name=f"b1{l}")
                nc.sync.dma_start(bq_t[:], dbq.ap()[l].rearrange("(c p) -> p c", p=P))
                nc.sync.dma_start(bk_t[:], dbk.ap()[l].rearrange("(c p) -> p c", p=P))
                nc.sync.dma_start(b1_t[:], db1.ap()[l].rearrange("(c p) -> p c", p=P))
                brow_t = wp.tile([1, 2 * H], bf16, name=f"brow{l}")
                nc.sync.dma_start(brow_t[:], dbrow.ap()[l])

                # single rotating pool for all weight chunks of this layer
                wpool = ls.enter_context(tc.tile_pool(name=f"w{l}", bufs=24))

                def wload(dW, r0, c0, cols=H):
                    w = wpool.tile([P, H], bf16, tag="wc")
                    nc.gpsimd.dma_start(w[:, :cols],
                                        dW.ap()[l, r0:r0 + P, c0:c0 + cols])
                    return w

                cur, nxt = (curA, curB)

                with ExitStack() as attn_scope:
                    ap_ = attn_scope.enter_context(
                        tc.tile_pool(name=f"attn{l}", bufs=1))
                    QT = [ap_.tile([P, NT], bf16, name=f"QT{l}_{h}") for h in range(HC)]
                    KT = [ap_.tile([P, NT], bf16, name=f"KT{l}_{h}") for h in range(HC)]
                    # V with an appended ones column per head (65 cols/head)
                    Vt = [ap_.tile([P, NH, DH + 1], bf16, name=f"V{l}_{t}")
                          for t in range(TC)]
                    ctxT = [ap_.tile([P, NT], bf16, name=f"cT{l}_{h}") for h in range(HC)]

                    wv_c = [wload(dWv, h * P, 0) for h in range(HC)]
                    wq_c = [wload(dWq, h * P, 0) for h in range(HC)]
                    wk_c = [wload(dWk, h * P, 0) for h in range(HC)]
                    wo_c = [wload(dWo, h * P, 0) for h in range(HC)]

                    def qk_group(dstT, wch, bt, ho, nf):
                        ps = psum.tile([P, S], f32, tag="a", bufs=2, name="psqk")
                        for hi in range(HC):
                            nc.tensor.matmul(
                                ps[:], lhsT=wch[hi][:, ho * P:(ho + 1) * P],
                                rhs=rhs_x(nf, hi),
                                start=(hi == 0), stop=(hi == HC - 1))
                        nc.vector.tensor_scalar(
                            out=dstT[ho][:, nf * S:(nf + 1) * S], in0=ps[:],
                            scalar1=bt[:, ho:ho + 1], scalar2=None, op0=OP.add)

                    def v_group(t):
                        pv = psum.tile([P, 2 * S], f32, tag="b", bufs=2, name="psv")
                        for nf, n0, nn in ((0, 0, S), (1, S, H - S)):
                            for hi in range(HC):
                                nc.tensor.matmul(
                                    pv[:, n0:n0 + nn],
                                    lhsT=xT[:, t, hi, :],
                                    rhs=wv_c[hi][:, n0:n0 + nn],
                                    start=(hi == 0), stop=(hi == HC - 1))
                        nc.vector.tensor_copy(
                            Vt[t][:, :, 0:DH],
                            pv[:, 0:H].rearrange("p (h d) -> p h d", h=NH))
                        nc.vector.memset(Vt[t][:, :, DH:DH + 1], 1.0)

                    # --- attention unit pieces (seq s, head hd) ---
                    ex_tiles = {}

                    def sc_unit(s, hd):
                        """scoresT + exp for one (seq, head): 4 kc tiles."""
                        hc, po = hd // 2, (hd % 2) * DH
                        exs = []
                        for kc in range(SC):
                            ps = psum.tile([P, S], f32, tag="a", bufs=2, name="pss")
                            nc.tensor.matmul(
                                ps[:],
                                lhsT=KT[hc][po:po + DH,
                                            s * S + kc * P:s * S + (kc + 1) * P],
                                rhs=QT[hc][po:po + DH, s * S:(s + 1) * S],
                                start=True, stop=True, tile_position=(po, 0))
                            ex = ap_.tile([P, S], bf16, tag="ex", bufs=16, name="ex")
                            if with_mask:
                                nc.scalar.activation(
                                    ex[:], ps[:], AF.Exp,
                                    bias=extm_sb[:, s * SC + kc:s * SC + kc + 1],
                                    scale=INV_SQRT_DH)
                            else:
                                nc.scalar.activation(ex[:], ps[:], AF.Exp,
                                                     scale=INV_SQRT_DH)
                            exs.append(ex)
                        ex_tiles[(s, hd)] = exs

                    def ctx_unit(s, hd):
                        """ctx + denominator + normalize for one (seq, head)."""
                        hc, po = hd // 2, (hd % 2) * DH
                        exs = ex_tiles.pop((s, hd))
                        cx = psum.tile([P, S], f32, tag="c", bufs=2, name="cx")
                        for kc in range(SC):
                            nc.tensor.matmul(
                                cx[0:DH + 1, :],
                                lhsT=Vt[s * SC + kc][:, hd, :],
                                rhs=exs[kc][:],
                                start=(kc == 0), stop=(kc == SC - 1))
                        row = small.tile([1, S], f32, tag="row", bufs=4)
                        nc.vector.tensor_copy(row[:], cx[DH:DH + 1, :])
                        rinv = small.tile([1, S], f32, tag="rinv", bufs=4)
                        # ~51 ULP is plenty (feeds bf16 math); sums of
                        # positive exps can't hit the undefined edge cases
                        nc.vector.reciprocal_approx_fast(out=rinv[:], in_=row[:])
                        rb = small.tile([DH, S], f32, tag="rb", bufs=3)
                        nc.gpsimd.partition_broadcast(rb[:], rinv[:], channels=DH)
                        nc.vector.tensor_tensor(
                            out=ctxT[hc][po:po + DH, s * S:(s + 1) * S],
                            in0=cx[0:DH, :], in1=rb[:], op=OP.mult)

                    def ao_group(t):
                        """attn-out projection + residual + LN1 for chunk t."""
                        po_ = psum.tile([P, 2 * S], f32, tag="b", bufs=2, name="pao")
                        for nf, n0, nn in ((0, 0, S), (1, S, H - S)):
                            for hi in range(HC):
                                nc.tensor.matmul(
                                    po_[:, n0:n0 + nn],
                                    lhsT=ctxT[hi][:, t * P:(t + 1) * P],
                                    rhs=wo_c[hi][:, n0:n0 + nn],
                                    start=(hi == 0),
                                    stop=(hi == HC - 1 and not with_brow))
                            if with_brow:
                                nc.tensor.matmul(po_[:, n0:n0 + nn], lhsT=ones1[:],
                                                 rhs=brow_t[0:1, n0:n0 + nn],
                                                 start=False, stop=True)
                        ln_store(po_[:, 0:H], cur[t][:], nxt[t], t)
                        transpose_t(t)

                    # --- emission schedule ---
                    # Only nf0 (seq0) xT chunks are ready at layer start (the
                    # previous layer's t4..7 LN2 transposes land a few us in),
                    # so everything nf1-dependent is deferred into the seq1
                    # stream.  V(seq0) first so seq0 ctx units can interleave
                    # with the QK/scores stream (ctx lags scores by 2 units:
                    # at most 4 score units = 16 ex tiles in flight).
                    for t in range(SC):
                        v_group(t)
                    for hc in range(HC):
                        qk_group(QT, wq_c, bq_t, hc, 0)
                        qk_group(KT, wk_c, bk_t, hc, 0)
                        if hc > 0:
                            ctx_unit(0, 2 * hc - 2)
                            ctx_unit(0, 2 * hc - 1)
                        sc_unit(0, 2 * hc)
                        sc_unit(0, 2 * hc + 1)
                    # seq1: nf1 QK groups + V(seq1) spread into the scores
                    # pipeline, ctx lags scores by 2
                    ctx_unit(0, NH - 2)
                    ctx_unit(0, NH - 1)
                    for h in range(NH):
                        if h % 2 == 0:
                            qk_group(QT, wq_c, bq_t, h // 2, 1)
                            qk_group(KT, wk_c, bk_t, h // 2, 1)
                        sc_unit(1, h)
                        if h < 2:
                            v_group(SC + 2 * h)
                            v_group(SC + 2 * h + 1)
                        if h >= 2:
                            ctx_unit(1, h - 2)
                        if h >= NH - SC:
                            ao_group(h - (NH - SC))  # t0..3 (seq0 ctx done)
                    # tail: remaining ctx + seq1 attn-out/LN1
                    ctx_unit(1, NH - 2)
                    ctx_unit(1, NH - 1)
                    if l == 0:
                        tap("QT", QT)
                        tap("KT", KT)
                        tap("ctxT", ctxT)
                    for t in range(SC, TC):
                        ao_group(t)
                    if l == 0:
                        tap("ln1", [nxt[t] for t in range(TC)])

                # --- FFN ---
                with ExitStack() as ffn_scope:
                    fp_ = ffn_scope.enter_context(
                        tc.tile_pool(name=f"ffn{l}", bufs=1))
                    gT = [fp_.tile([P, NT], bf16, name=f"gT{l}_{i}") for i in range(IC)]
                    # W1 split into [128,768] chunks in the rotating pool
                    w1_c = [[wload(dW1, hi * P, j * H) for j in range(IC // HC)]
                            for hi in range(HC)]

                    # nf-outer: the 24 nf0 groups cover the latency of the
                    # seq1 LN1 chains + transposes that nf1 depends on
                    for nf in range(2):
                        for i_ in range(IC):
                            ps = psum.tile([P, S], f32, tag="a", bufs=2, name="psf")
                            for hi in range(HC):
                                w1t = w1_c[hi][(i_ * P) // H]
                                c0 = (i_ * P) % H
                                nc.tensor.matmul(
                                    ps[:], lhsT=w1t[:, c0:c0 + P],
                                    rhs=rhs_x(nf, hi),
                                    start=(hi == 0), stop=(hi == HC - 1))
                            nc.scalar.activation(
                                gT[i_][:, nf * S:(nf + 1) * S], ps[:], AF.Gelu,
                                bias=b1_t[:, i_:i_ + 1], scale=1.0)
                    if l == 0:
                        tap("gT", gT)

                    w2_c = [wload(dW2, i_ * P, 0) for i_ in range(IC)]
                    for t in range(TC):
                        pf = psum.tile([P, 2 * S], f32, tag="b", bufs=2, name="pf")
                        for i_ in range(IC):
                            for nf, n0, nn in ((0, 0, S), (1, S, H - S)):
                                nc.tensor.matmul(
                                    pf[:, n0:n0 + nn],
                                    lhsT=gT[i_][:, t * P:(t + 1) * P],
                                    rhs=w2_c[i_][:, n0:n0 + nn],
                                    start=(i_ == 0),
                                    stop=(i_ == IC - 1 and not with_brow))
                        if with_brow:
                            for nf, n0, nn in ((0, 0, S), (1, S, H - S)):
                                nc.tensor.matmul(pf[:, n0:n0 + nn], lhsT=ones1[:],
                                                 rhs=brow_t[0:1, H + n0:H + n0 + nn],
                                                 start=False, stop=True)
                        last = (l == layers - 1)
                        ln_store(pf[:, 0:H], nxt[t][:], cur[t], t, last=last,
                                 out_f32=out.ap()[t * P:(t + 1) * P, :])
                        if not last:
                            transpose_t(t)

    nc.compile()
    return nc


# --------------------------------------------------------------------------
# host side
# --------------------------------------------------------------------------

def prep_shared(inputs):
    sh = {}
    sh["wemb_bf"] = inputs["word_emb"].astype(_BF16)
    sh["temb_bf"] = inputs["type_emb"].astype(_BF16)
    sh["pemb"] = inputs["pos_emb"].astype(np.float32)
    for k in ("Wq", "Wk", "Wv", "Wo", "W1", "W2"):
        sh[k] = inputs[k].astype(_BF16)
    sh["bq"] = inputs["bq"].astype(np.float32)
    sh["bk"] = inputs["bk"].astype(np.float32)
    sh["b1"] = inputs["b1"].astype(np.float32)
    # brow rows: [bv @ Wo + bo, b2]  (bv folded through the out-projection)
    bvWo = np.einsum("lh,lho->lo", inputs["bv"].astype(np.float64),
                     inputs["Wo"].astype(np.float64)).astype(np.float32)
    browo = (bvWo + inputs["bo"]).astype(np.float32)
    sh["brow"] = np.concatenate([browo, inputs["b2"]],
                                axis=1)[:, None, :].astype(_BF16)
    return sh


def core_inputs(inputs, sh, c):
    ids = np.asarray(inputs["input_ids"]).astype(np.int64)
    tts = np.asarray(inputs["token_type_ids"]).astype(np.int64)
    am = np.asarray(inputs["attention_mask"]).astype(np.float32)
    b0 = c * B_LOC
    m = {k: v for k, v in sh.items() if k not in ("wemb_bf", "temb_bf")}
    m["wrows"] = np.ascontiguousarray(sh["wemb_bf"][ids[b0:b0 + B_LOC].reshape(-1)])
    m["trows"] = np.ascontiguousarray(sh["temb_bf"][tts[b0:b0 + B_LOC].reshape(-1)])
    # [P, B_LOC*SC] per-partition mask columns
    em = ((1.0 - am[b0:b0 + B_LOC]) * -10000.0).reshape(B_LOC * SC, P)
    m["extmc"] = np.ascontiguousarray(em.T).astype(np.float32)
    return m


_NC_CACHE = {}


def flags_for(inputs):
    with_mask = not np.all(np.asarray(inputs["attention_mask"]) == 1.0)
    with_brow = bool(np.any(np.asarray(inputs["bo"])) or
                     np.any(np.asarray(inputs["b2"])) or
                     np.any(np.asarray(inputs["bv"])))
    return with_mask, with_brow


def get_nc(layers=L, with_mask=False, with_brow=False):
    key = (layers, with_mask, with_brow)
    if key not in _NC_CACHE:
        _NC_CACHE[key] = build(layers, with_mask=with_mask, with_brow=with_brow)
    return _NC_CACHE[key]


def run(inputs, layers=L):
    from concourse.bass_utils import run_bass_kernel_spmd
    inputs = {k: np.asarray(v) for k, v in inputs.items()}
    wm, wb = flags_for(inputs)
    nc = get_nc(layers, wm, wb)
    sh = prep_shared(inputs)
    in_maps = [core_inputs(inputs, sh, c) for c in range(NCORES)]
    res = run_bass_kernel_spmd(nc, in_maps, core_ids=list(range(NCORES)))
    outs = [res.results[c]["out"].reshape(B_LOC, S, H) for c in range(NCORES)]
    return np.concatenate(outs, axis=0).astype(np.float32)


def kernel(**inputs):
    return run(inputs)


# revision 19
# speedup vs baseline: 1.2660x; 1.1114x over previous
"""BERT-base forward on 8 Trainium2 NeuronCores.

Strategy: pure data parallelism over the batch (B=16 -> 2 sequences per
core), weights replicated, zero collectives.

v6 redesign (from v5's trace: ACT 52%/2.4ms + DVE 36%/1.66ms serialized
against PE 85%/3.9ms, 29% of the span HAM-throttled at half clock):

* k-major attention: scores are computed transposed ([k, q]) so the
  softmax probabilities feed the ctx matmul directly -- the 384 PE
  transpose-matmuls + 96 psum copies per layer of v5 are gone.  The
  softmax denominator comes for free as row 64 of the ctx matmul
  (V carries an appended ones column, M=65); normalization happens
  after ctx via DVE-recip -> gpsimd partition-broadcast -> DVE mult.
* token->feature layout changes (xtok -> xT) run on the DMA xbar
  transpose engine instead of PE matmul + ACT copy.
* PSUM evacuations moved from ACT to DVE; ACT keeps exp/gelu/sqrt and
  the bf16 xtok copies only.
* 1/sqrt(DH) folded into the exp scale; bv folded into the attn-out
  row bias host-side (brow_o = bv @ Wo + bo); mask folded into the
  exp per-partition bias (k-major makes the mask per-partition).
* single rotating weight-chunk pool ([128,768] bf16 x 24 bufs) streams
  Wq/Wk/Wv/Wo/W1/W2 and prefetches across phases.

Layout conventions per core (P=128 partitions):
  tokens NT=1024 (2 seqs x 512), token chunk t in [0,8)
  token-major  [128 tokens, H]   - residual stream, layernorm
  feature-major xT [128, t, hc, 128] - matmul lhsT/rhs operands
  matmul computes out = lhsT.T @ rhs (contraction along partitions)
"""

import numpy as np
import ml_dtypes

V, H, L, NH, I, S = 30522, 768, 12, 12, 3072, 512
B_FULL, NCORES, B_LOC = 16, 8, 2
DH = H // NH                      # 64
P = 128
NT = B_LOC * S                    # 1024 tokens per core
TC = NT // P                      # 8 token chunks
HC = H // P                       # 6 feature chunks
IC = I // P                       # 24 ffn chunks
SC = S // P                       # 4 chunks per sequence
EPS = 1e-12
INV_SQRT_DH = 1.0 / 8.0

_BF16 = ml_dtypes.bfloat16


# --------------------------------------------------------------------------
# device kernel builder
# --------------------------------------------------------------------------

def build(layers=L, taps=None, with_mask=False, with_brow=False):
    import concourse.bass as bass
    import concourse.mybir as mybir
    import concourse.tile as tile
    from concourse import bacc
    from contextlib import ExitStack

    dt = mybir.dt
    AF = mybir.ActivationFunctionType
    OP = mybir.AluOpType

    nc = bacc.Bacc("TRN2", target_bir_lowering=False, debug=False,
                   num_devices=NCORES)

    # ---- DRAM inputs (per core) ----
    wrows = nc.dram_tensor("wrows", [NT, H], dt.bfloat16, kind="ExternalInput")
    trows = nc.dram_tensor("trows", [NT, H], dt.bfloat16, kind="ExternalInput")
    pemb = nc.dram_tensor("pemb", [S, H], dt.float32, kind="ExternalInput")
    # mask as per-partition columns: extmc[p, s*SC+kc] = mask bias of
    # k-token s*512 + kc*128 + p (k-major scores make the mask per-partition)
    extmc = nc.dram_tensor("extmc", [P, B_LOC * SC], dt.float32,
                           kind="ExternalInput")
    dWq = nc.dram_tensor("Wq", [L, H, H], dt.bfloat16, kind="ExternalInput")
    dWk = nc.dram_tensor("Wk", [L, H, H], dt.bfloat16, kind="ExternalInput")
    dWv = nc.dram_tensor("Wv", [L, H, H], dt.bfloat16, kind="ExternalInput")
    dWo = nc.dram_tensor("Wo", [L, H, H], dt.bfloat16, kind="ExternalInput")
    dW1 = nc.dram_tensor("W1", [L, H, I], dt.bfloat16, kind="ExternalInput")
    dW2 = nc.dram_tensor("W2", [L, I, H], dt.bfloat16, kind="ExternalInput")
    dbq = nc.dram_tensor("bq", [L, H], dt.float32, kind="ExternalInput")
    dbk = nc.dram_tensor("bk", [L, H], dt.float32, kind="ExternalInput")
    db1 = nc.dram_tensor("b1", [L, I], dt.float32, kind="ExternalInput")
    # free-dim biases (added via K=1 rank-1 matmuls): rows [bv@Wo+bo, b2]
    dbrow = nc.dram_tensor("brow", [L, 1, 2 * H], dt.bfloat16, kind="ExternalInput")
    out = nc.dram_tensor("out", [NT, H], dt.float32, kind="ExternalOutput")

    f32, bf16 = dt.float32, dt.bfloat16

    def tap(name, tiles):
        if taps is None:
            return
        sh0 = list(tiles[0].shape)
        d = nc.dram_tensor(f"tap_{name}", [len(tiles)] + sh0,
                           tiles[0].dtype, kind="ExternalOutput")
        for i, t in enumerate(tiles):
            nc.sync.dma_start(d.ap()[i], t[:])
        taps[name] = d

    with tile.TileContext(nc) as tc_, ExitStack() as top:
        tc = tc_

        # ---- constants & persistent activation tiles ----
        pers = top.enter_context(tc.tile_pool(name="pers", bufs=1))
        ones1 = pers.tile([1, P], bf16, name="ones1")
        nc.vector.memset(ones1[:], 1.0)
        eps_t = pers.tile([P, 1], f32, name="eps_t")
        nc.vector.memset(eps_t[:], EPS)
        extm_sb = pers.tile([P, B_LOC * SC], f32, name="extm_sb")
        nc.sync.dma_start(extm_sb[:], extmc.ap())

        curA = [pers.tile([P, H], f32, name=f"curA{t}") for t in range(TC)]
        curB = [pers.tile([P, H], f32, name=f"curB{t}") for t in range(TC)]
        xtok = [pers.tile([P, H], bf16, name=f"xtok{t}") for t in range(TC)]
        # feature-major activations: xT[p, t, hc, j] = x[token t*128+j,
        # feature hc*128+p], written by DMA xbar transpose per t-chunk
        xT = pers.tile([P, TC, HC, P], bf16, name="xT")

        small = top.enter_context(tc.tile_pool(name="small", bufs=6))
        psum = top.enter_context(tc.tile_pool(name="psum", space="PSUM", bufs=1))
        # psum tags: "a" scores/QK/FFN1 [128,512]x2; "c" ctx [128,512]x2;
        # "b" V/attn-out/FFN2 [128,1024]x2 (768 used).  2+2+4 banks = 8.

        def rhs_x(nf, hi):
            """feature-major rhs [128, 512] for sequence-half nf, chunk hi."""
            return xT[:, 4 * nf:4 * nf + 4, hi, :]

        # ---------------- helpers ----------------
        def ln_store(src_ap, res_ap, dst, tcid, last=False, out_f32=None):
            """dst = layernorm(src + res); also writes bf16 copy to xtok[tcid]
            unless last (then DMAs fp32 to out_f32)."""
            s1 = small.tile([P, 1], f32, tag="s1")
            nc.vector.scalar_tensor_tensor(
                out=dst[:], in0=src_ap, scalar=0.0, in1=res_ap,
                op0=OP.add, op1=OP.add, accum_out=s1[:])
            u = small.tile([P, 1], f32, tag="u")
            nc.vector.tensor_scalar(out=u[:], in0=s1[:], scalar1=1.0 / H,
                                    scalar2=None, op0=OP.mult)
            junk = small.tile([P, H], f32, tag="junk", bufs=2)
            s2 = small.tile([P, 1], f32, tag="s2")
            nc.vector.scalar_tensor_tensor(
                out=junk[:], in0=dst[:], scalar=u[:], in1=dst[:],
                op0=OP.subtract, op1=OP.mult, accum_out=s2[:])
            sd = small.tile([P, 1], f32, tag="sd")
            nc.scalar.activation(sd[:], s2[:], AF.Sqrt, bias=eps_t[:], scale=1.0 / H)
            rstd = small.tile([P, 1], f32, tag="rstd")
            nc.vector.reciprocal(rstd[:], sd[:])
            nc.vector.tensor_scalar(out=dst[:], in0=dst[:], scalar1=u[:],
                                    scalar2=rstd[:], op0=OP.subtract, op1=OP.mult)
            if last:
                nc.scalar.dma_start(out_f32, dst[:])
            else:
                nc.scalar.copy(xtok[tcid][:], dst[:])

        def transpose_t(t):
            """xtok[t] (token-major bf16) -> xT[:, t] via DMA xbar.

            On the sync HWDGE queue; weight loads go through the gpsimd
            SWDGE queue so a transpose waiting on its xtok copy never
            head-of-line-blocks weight prefetch (queues are FIFO)."""
            nc.sync.dma_start_transpose(xT[:, t], xtok[t][:])

        # ---- embedding: gather + add + LN ----
        with ExitStack() as emb_scope:
            ep = emb_scope.enter_context(tc.tile_pool(name="emb", bufs=1))
            wg = ep.tile([P, TC, H], bf16, name="wg")
            tg = ep.tile([P, TC, H], bf16, name="tg")
            nc.sync.dma_start(wg[:], wrows.ap().rearrange("(c p) h -> p c h", p=P))
            nc.sync.dma_start(tg[:], trows.ap().rearrange("(c p) h -> p c h", p=P))
            pos = ep.tile([P, SC, H], f32, name="pos")
            nc.sync.dma_start(pos[:], pemb.ap().rearrange("(c p) h -> p c h", p=P))
            for t in range(TC):
                tmp = ep.tile([P, H], f32, tag="etmp", bufs=2, name="etmp")
                nc.vector.tensor_tensor(out=tmp[:], in0=tg[:, t],
                                        in1=pos[:, t % SC], op=OP.add)
                ln_store(wg[:, t], tmp[:], curA[t], t)
                transpose_t(t)
            tap("emb", curA)

        # ---- transformer layers ----
        for l in range(layers):
            with ExitStack() as ls:
                wp = ls.enter_context(tc.tile_pool(name=f"bias{l}", bufs=1))
                bq_t = wp.tile([P, HC], f32, name=f"bq{l}")
                bk_t = wp.tile([P, HC], f32, name=f"bk{l}")
                b1_t = wp.tile([P, IC], f32, name=f"b1{l}")
                nc.sync.dma_start(bq_t[:], dbq.ap()[l].rearrange("(c p) -> p c", p=P))
                nc.sync.dma_start(bk_t[:], dbk.ap()[l].rearrange("(c p) -> p c", p=P))
                nc.sync.dma_start(b1_t[:], db1.ap()[l].rearrange("(c p) -> p c", p=P))
                brow_t = wp.tile([1, 2 * H], bf16, name=f"brow{l}")
                nc.sync.dma_start(brow_t[:], dbrow.ap()[l])

                # single rotating pool for all weight chunks of this layer
                wpool = ls.enter_context(tc.tile_pool(name=f"w{l}", bufs=27))

                def wload(dW, r0, c0, cols=H):
                    w = wpool.tile([P, H], bf16, tag="wc")
                    nc.gpsimd.dma_start(w[:, :cols],
                                        dW.ap()[l, r0:r0 + P, c0:c0 + cols])
                    return w

                cur, nxt = (curA, curB)

                with ExitStack() as attn_scope:
                    ap_ = attn_scope.enter_context(
                        tc.tile_pool(name=f"attn{l}", bufs=1))
                    QT = [ap_.tile([P, NT], bf16, name=f"QT{l}_{h}") for h in range(HC)]
                    KT = [ap_.tile([P, NT], bf16, name=f"KT{l}_{h}") for h in range(HC)]
                    # V with an appended ones column per head (65 cols/head)
                    Vt = [ap_.tile([P, NH, DH + 1], bf16, name=f"V{l}_{t}")
                          for t in range(TC)]
                    ctxT = [ap_.tile([P, NT], bf16, name=f"cT{l}_{h}") for h in range(HC)]

                    wv_c = [wload(dWv, h * P, 0) for h in range(HC)]
                    wq_c = [wload(dWq, h * P, 0) for h in range(HC)]
                    wk_c = [wload(dWk, h * P, 0) for h in range(HC)]
                    wo_c = [wload(dWo, h * P, 0) for h in range(HC)]
                    # W1 loads issued here (not at FFN emission) so the SWDGE
                    # dispatches aren't queued behind attention's broadcasts
                    w1_c = [[wload(dW1, hi * P, j * H) for j in range(IC // HC)]
                            for hi in range(HC)]

                    def qk_group(dstT, wch, bt, ho, nf):
                        ps = psum.tile([P, S], f32, tag="a", bufs=2, name="psqk")
                        for hi in range(HC):
                            nc.tensor.matmul(
                                ps[:], lhsT=wch[hi][:, ho * P:(ho + 1) * P],
                                rhs=rhs_x(nf, hi),
                                start=(hi == 0), stop=(hi == HC - 1))
                        nc.vector.tensor_scalar(
                            out=dstT[ho][:, nf * S:(nf + 1) * S], in0=ps[:],
                            scalar1=bt[:, ho:ho + 1], scalar2=None, op0=OP.add)

                    def v_group(t):
                        pv = psum.tile([P, 2 * S], f32, tag="b", bufs=2, name="psv")
                        for nf, n0, nn in ((0, 0, S), (1, S, H - S)):
                            for hi in range(HC):
                                nc.tensor.matmul(
                                    pv[:, n0:n0 + nn],
                                    lhsT=xT[:, t, hi, :],
                                    rhs=wv_c[hi][:, n0:n0 + nn],
                                    start=(hi == 0), stop=(hi == HC - 1))
                        nc.vector.tensor_copy(
                            Vt[t][:, :, 0:DH],
                            pv[:, 0:H].rearrange("p (h d) -> p h d", h=NH))
                        nc.vector.memset(Vt[t][:, :, DH:DH + 1], 1.0)

                    # --- attention unit pieces (seq s, head hd) ---
                    ex_tiles = {}

                    def sc_unit(s, hd):
                        """scoresT + exp for one (seq, head): 4 kc tiles."""
                        hc, po = hd // 2, (hd % 2) * DH
                        exs = []
                        for kc in range(SC):
                            ps = psum.tile([P, S], f32, tag="a", bufs=2, name="pss")
                            nc.tensor.matmul(
                                ps[:],
                                lhsT=KT[hc][po:po + DH,
                                            s * S + kc * P:s * S + (kc + 1) * P],
                                rhs=QT[hc][po:po + DH, s * S:(s + 1) * S],
                                start=True, stop=True, tile_position=(po, 0))
                            ex = ap_.tile([P, S], bf16, tag="ex", bufs=14, name="ex")
                            if with_mask:
                                nc.scalar.activation(
                                    ex[:], ps[:], AF.Exp,
                                    bias=extm_sb[:, s * SC + kc:s * SC + kc + 1],
                                    scale=INV_SQRT_DH)
                            else:
                                nc.scalar.activation(ex[:], ps[:], AF.Exp,
                                                     scale=INV_SQRT_DH)
                            exs.append(ex)
                        ex_tiles[(s, hd)] = exs

                    def ctx_unit(s, hd):
                        """ctx + denominator + normalize for one (seq, head)."""
                        hc, po = hd // 2, (hd % 2) * DH
                        exs = ex_tiles.pop((s, hd))
                        cx = psum.tile([P, S], f32, tag="c", bufs=2, name="cx")
                        for kc in range(SC):
                            nc.tensor.matmul(
                                cx[0:DH + 1, :],
                                lhsT=Vt[s * SC + kc][:, hd, :],
                                rhs=exs[kc][:],
                                start=(kc == 0), stop=(kc == SC - 1))
                        row = small.tile([1, S], f32, tag="row", bufs=4)
                        nc.vector.tensor_copy(row[:], cx[DH:DH + 1, :])
                        rinv = small.tile([1, S], f32, tag="rinv", bufs=4)
                        # ~51 ULP is plenty (feeds bf16 math); sums of
                        # positive exps can't hit the undefined edge cases
                        nc.vector.reciprocal_approx_fast(out=rinv[:], in_=row[:])
                        rb = small.tile([DH, S], f32, tag="rb", bufs=3)
                        nc.gpsimd.partition_broadcast(rb[:], rinv[:], channels=DH)
                        nc.vector.tensor_tensor(
                            out=ctxT[hc][po:po + DH, s * S:(s + 1) * S],
                            in0=cx[0:DH, :], in1=rb[:], op=OP.mult)

                    def ao_group(t):
                        """attn-out projection + residual + LN1 for chunk t."""
                        po_ = psum.tile([P, 2 * S], f32, tag="b", bufs=2, name="pao")
                        for nf, n0, nn in ((0, 0, S), (1, S, H - S)):
                            for hi in range(HC):
                                nc.tensor.matmul(
                                    po_[:, n0:n0 + nn],
                                    lhsT=ctxT[hi][:, t * P:(t + 1) * P],
                                    rhs=wo_c[hi][:, n0:n0 + nn],
                                    start=(hi == 0),
                                    stop=(hi == HC - 1 and not with_brow))
                            if with_brow:
                                nc.tensor.matmul(po_[:, n0:n0 + nn], lhsT=ones1[:],
                                                 rhs=brow_t[0:1, n0:n0 + nn],
                                                 start=False, stop=True)
                        ln_store(po_[:, 0:H], cur[t][:], nxt[t], t)
                        transpose_t(t)

                    # --- emission schedule ---
                    # Only nf0 (seq0) xT chunks are ready at layer start (the
                    # previous layer's t4..7 LN2 transposes land a few us in),
                    # so everything nf1-dependent is deferred into the seq1
                    # stream.  V(seq0) first so seq0 ctx units can interleave
                    # with the QK/scores stream (ctx lags scores by 2 units:
                    # at most 4 score units = 16 ex tiles in flight).
                    for t in range(SC):
                        v_group(t)
                    for hc in range(HC):
                        qk_group(QT, wq_c, bq_t, hc, 0)
                        qk_group(KT, wk_c, bk_t, hc, 0)
                        if hc > 0:
                            ctx_unit(0, 2 * hc - 2)
                            ctx_unit(0, 2 * hc - 1)
                        sc_unit(0, 2 * hc)
                        sc_unit(0, 2 * hc + 1)
                    # seq1: nf1 QK groups + V(seq1) spread into the scores
                    # pipeline, ctx lags scores by 2
                    ctx_unit(0, NH - 2)
                    ctx_unit(0, NH - 1)
                    for h in range(NH):
                        if h % 2 == 0:
                            qk_group(QT, wq_c, bq_t, h // 2, 1)
                            qk_group(KT, wk_c, bk_t, h // 2, 1)
                        sc_unit(1, h)
                        if h < 2:
                            v_group(SC + 2 * h)
                            v_group(SC + 2 * h + 1)
                        if h >= 2:
                            ctx_unit(1, h - 2)
                        if h >= NH - SC:
                            ao_group(h - (NH - SC))  # t0..3 (seq0 ctx done)
                    # tail: remaining ctx + seq1 attn-out/LN1
                    ctx_unit(1, NH - 2)
                    ctx_unit(1, NH - 1)
                    if l == 0:
                        tap("QT", QT)
                        tap("KT", KT)
                        tap("ctxT", ctxT)
                    for t in range(SC, TC):
                        ao_group(t)
                    if l == 0:
                        tap("ln1", [nxt[t] for t in range(TC)])

                # --- FFN ---
                with ExitStack() as ffn_scope:
                    fp_ = ffn_scope.enter_context(
                        tc.tile_pool(name=f"ffn{l}", bufs=1))
                    gT = [fp_.tile([P, NT], bf16, name=f"gT{l}_{i}") for i in range(IC)]

                    # nf-outer: the 24 nf0 groups cover the latency of the
                    # seq1 LN1 chains + transposes that nf1 depends on
                    for nf in range(2):
                        for i_ in range(IC):
                            ps = psum.tile([P, S], f32, tag="a", bufs=2, name="psf")
                            for hi in range(HC):
                                w1t = w1_c[hi][(i_ * P) // H]
                                c0 = (i_ * P) % H
                                nc.tensor.matmul(
                                    ps[:], lhsT=w1t[:, c0:c0 + P],
                                    rhs=rhs_x(nf, hi),
                                    start=(hi == 0), stop=(hi == HC - 1))
                            nc.scalar.activation(
                                gT[i_][:, nf * S:(nf + 1) * S], ps[:], AF.Gelu,
                                bias=b1_t[:, i_:i_ + 1], scale=1.0)
                    if l == 0:
                        tap("gT", gT)

                    w2_c = [wload(dW2, i_ * P, 0) for i_ in range(IC)]
                    for t in range(TC):
                        pf = psum.tile([P, 2 * S], f32, tag="b", bufs=2, name="pf")
                        for i_ in range(IC):
                            for nf, n0, nn in ((0, 0, S), (1, S, H - S)):
                                nc.tensor.matmul(
                                    pf[:, n0:n0 + nn],
                                    lhsT=gT[i_][:, t * P:(t + 1) * P],
                                    rhs=w2_c[i_][:, n0:n0 + nn],
                                    start=(i_ == 0),
                                    stop=(i_ == IC - 1 and not with_brow))
                        if with_brow:
                            for nf, n0, nn in ((0, 0, S), (1, S, H - S)):
                                nc.tensor.matmul(pf[:, n0:n0 + nn], lhsT=ones1[:],
                                                 rhs=brow_t[0:1, H + n0:H + n0 + nn],
                                                 start=False, stop=True)
                        last = (l == layers - 1)
                        ln_store(pf[:, 0:H], nxt[t][:], cur[t], t, last=last,
                                 out_f32=out.ap()[t * P:(t + 1) * P, :])
                        if not last:
                            transpose_t(t)

    nc.compile()
    return nc


# --------------------------------------------------------------------------
# host side
# --------------------------------------------------------------------------

def prep_shared(inputs):
    sh = {}
    sh["wemb_bf"] = inputs["word_emb"].astype(_BF16)
    sh["temb_bf"] = inputs["type_emb"].astype(_BF16)
    sh["pemb"] = inputs["pos_emb"].astype(np.float32)
    for k in ("Wq", "Wk", "Wv", "Wo", "W1", "W2"):
        sh[k] = inputs[k].astype(_BF16)
    sh["bq"] = inputs["bq"].astype(np.float32)
    sh["bk"] = inputs["bk"].astype(np.float32)
    sh["b1"] = inputs["b1"].astype(np.float32)
    # brow rows: [bv @ Wo + bo, b2]  (bv folded through the out-projection)
    bvWo = np.einsum("lh,lho->lo", inputs["bv"].astype(np.float64),
                     inputs["Wo"].astype(np.float64)).astype(np.float32)
    browo = (bvWo + inputs["bo"]).astype(np.float32)
    sh["brow"] = np.concatenate([browo, inputs["b2"]],
                                axis=1)[:, None, :].astype(_BF16)
    return sh


def core_inputs(inputs, sh, c):
    ids = np.asarray(inputs["input_ids"]).astype(np.int64)
    tts = np.asarray(inputs["token_type_ids"]).astype(np.int64)
    am = np.asarray(inputs["attention_mask"]).astype(np.float32)
    b0 = c * B_LOC
    m = {k: v for k, v in sh.items() if k not in ("wemb_bf", "temb_bf")}
    m["wrows"] = np.ascontiguousarray(sh["wemb_bf"][ids[b0:b0 + B_LOC].reshape(-1)])
    m["trows"] = np.ascontiguousarray(sh["temb_bf"][tts[b0:b0 + B_LOC].reshape(-1)])
    # [P, B_LOC*SC] per-partition mask columns
    em = ((1.0 - am[b0:b0 + B_LOC]) * -10000.0).reshape(B_LOC * SC, P)
    m["extmc"] = np.ascontiguousarray(em.T).astype(np.float32)
    return m


_NC_CACHE = {}


def flags_for(inputs):
    with_mask = not np.all(np.asarray(inputs["attention_mask"]) == 1.0)
    with_brow = bool(np.any(np.asarray(inputs["bo"])) or
                     np.any(np.asarray(inputs["b2"])) or
                     np.any(np.asarray(inputs["bv"])))
    return with_mask, with_brow


def get_nc(layers=L, with_mask=False, with_brow=False):
    key = (layers, with_mask, with_brow)
    if key not in _NC_CACHE:
        _NC_CACHE[key] = build(layers, with_mask=with_mask, with_brow=with_brow)
    return _NC_CACHE[key]


def run(inputs, layers=L):
    from concourse.bass_utils import run_bass_kernel_spmd
    inputs = {k: np.asarray(v) for k, v in inputs.items()}
    wm, wb = flags_for(inputs)
    nc = get_nc(layers, wm, wb)
    sh = prep_shared(inputs)
    in_maps = [core_inputs(inputs, sh, c) for c in range(NCORES)]
    res = run_bass_kernel_spmd(nc, in_maps, core_ids=list(range(NCORES)))
    outs = [res.results[c]["out"].reshape(B_LOC, S, H) for c in range(NCORES)]
    return np.concatenate(outs, axis=0).astype(np.float32)


def kernel(**inputs):
    return run(inputs)


# revision 20
# speedup vs baseline: 1.2742x; 1.0065x over previous
"""BERT-base forward on 8 Trainium2 NeuronCores.

Strategy: pure data parallelism over the batch (B=16 -> 2 sequences per
core), weights replicated, zero collectives.

v6 redesign (from v5's trace: ACT 52%/2.4ms + DVE 36%/1.66ms serialized
against PE 85%/3.9ms, 29% of the span HAM-throttled at half clock):

* k-major attention: scores are computed transposed ([k, q]) so the
  softmax probabilities feed the ctx matmul directly -- the 384 PE
  transpose-matmuls + 96 psum copies per layer of v5 are gone.  The
  softmax denominator comes for free as row 64 of the ctx matmul
  (V carries an appended ones column, M=65); normalization happens
  after ctx via DVE-recip -> gpsimd partition-broadcast -> DVE mult.
* token->feature layout changes (xtok -> xT) run on the DMA xbar
  transpose engine instead of PE matmul + ACT copy.
* PSUM evacuations moved from ACT to DVE; ACT keeps exp/gelu/sqrt and
  the bf16 xtok copies only.
* 1/sqrt(DH) folded into the exp scale; bv folded into the attn-out
  row bias host-side (brow_o = bv @ Wo + bo); mask folded into the
  exp per-partition bias (k-major makes the mask per-partition).
* single rotating weight-chunk pool ([128,768] bf16 x 24 bufs) streams
  Wq/Wk/Wv/Wo/W1/W2 and prefetches across phases.

Layout conventions per core (P=128 partitions):
  tokens NT=1024 (2 seqs x 512), token chunk t in [0,8)
  token-major  [128 tokens, H]   - residual stream, layernorm
  feature-major xT [128, t, hc, 128] - matmul lhsT/rhs operands
  matmul computes out = lhsT.T @ rhs (contraction along partitions)
"""

import numpy as np
import ml_dtypes

V, H, L, NH, I, S = 30522, 768, 12, 12, 3072, 512
B_FULL, NCORES, B_LOC = 16, 8, 2
DH = H // NH                      # 64
P = 128
NT = B_LOC * S                    # 1024 tokens per core
TC = NT // P                      # 8 token chunks
HC = H // P                       # 6 feature chunks
IC = I // P                       # 24 ffn chunks
SC = S // P                       # 4 chunks per sequence
EPS = 1e-12
INV_SQRT_DH = 1.0 / 8.0

_BF16 = ml_dtypes.bfloat16


# --------------------------------------------------------------------------
# device kernel builder
# --------------------------------------------------------------------------

def build(layers=L, taps=None, with_mask=False, with_brow=False):
    import concourse.bass as bass
    import concourse.mybir as mybir
    import concourse.tile as tile
    from concourse import bacc
    from contextlib import ExitStack

    dt = mybir.dt
    AF = mybir.ActivationFunctionType
    OP = mybir.AluOpType

    nc = bacc.Bacc("TRN2", target_bir_lowering=False, debug=False,
                   num_devices=NCORES)

    # ---- DRAM inputs (per core) ----
    wrows = nc.dram_tensor("wrows", [NT, H], dt.bfloat16, kind="ExternalInput")
    trows = nc.dram_tensor("trows", [NT, H], dt.bfloat16, kind="ExternalInput")
    pemb = nc.dram_tensor("pemb", [S, H], dt.float32, kind="ExternalInput")
    # mask as per-partition columns: extmc[p, s*SC+kc] = mask bias of
    # k-token s*512 + kc*128 + p (k-major scores make the mask per-partition)
    extmc = nc.dram_tensor("extmc", [P, B_LOC * SC], dt.float32,
                           kind="ExternalInput")
    dWq = nc.dram_tensor("Wq", [L, H, H], dt.bfloat16, kind="ExternalInput")
    dWk = nc.dram_tensor("Wk", [L, H, H], dt.bfloat16, kind="ExternalInput")
    dWv = nc.dram_tensor("Wv", [L, H, H], dt.bfloat16, kind="ExternalInput")
    dWo = nc.dram_tensor("Wo", [L, H, H], dt.bfloat16, kind="ExternalInput")
    dW1 = nc.dram_tensor("W1", [L, H, I], dt.bfloat16, kind="ExternalInput")
    dW2 = nc.dram_tensor("W2", [L, I, H], dt.bfloat16, kind="ExternalInput")
    dbq = nc.dram_tensor("bq", [L, H], dt.float32, kind="ExternalInput")
    dbk = nc.dram_tensor("bk", [L, H], dt.float32, kind="ExternalInput")
    db1 = nc.dram_tensor("b1", [L, I], dt.float32, kind="ExternalInput")
    # free-dim biases (added via K=1 rank-1 matmuls): rows [bv@Wo+bo, b2]
    dbrow = nc.dram_tensor("brow", [L, 1, 2 * H], dt.bfloat16, kind="ExternalInput")
    out = nc.dram_tensor("out", [NT, H], dt.float32, kind="ExternalOutput")

    f32, bf16 = dt.float32, dt.bfloat16

    def tap(name, tiles):
        if taps is None:
            return
        sh0 = list(tiles[0].shape)
        d = nc.dram_tensor(f"tap_{name}", [len(tiles)] + sh0,
                           tiles[0].dtype, kind="ExternalOutput")
        for i, t in enumerate(tiles):
            nc.sync.dma_start(d.ap()[i], t[:])
        taps[name] = d

    with tile.TileContext(nc) as tc_, ExitStack() as top:
        tc = tc_

        # ---- constants & persistent activation tiles ----
        pers = top.enter_context(tc.tile_pool(name="pers", bufs=1))
        ones1 = pers.tile([1, P], bf16, name="ones1")
        nc.vector.memset(ones1[:], 1.0)
        eps_t = pers.tile([P, 1], f32, name="eps_t")
        nc.vector.memset(eps_t[:], EPS)
        extm_sb = pers.tile([P, B_LOC * SC], f32, name="extm_sb")
        nc.sync.dma_start(extm_sb[:], extmc.ap())

        curA = [pers.tile([P, H], f32, name=f"curA{t}") for t in range(TC)]
        curB = [pers.tile([P, H], f32, name=f"curB{t}") for t in range(TC)]
        xtok = [pers.tile([P, H], bf16, name=f"xtok{t}") for t in range(TC)]
        # feature-major activations: xT[p, t, hc, j] = x[token t*128+j,
        # feature hc*128+p], written by DMA xbar transpose per t-chunk
        xT = pers.tile([P, TC, HC, P], bf16, name="xT")

        small = top.enter_context(tc.tile_pool(name="small", bufs=6))
        psum = top.enter_context(tc.tile_pool(name="psum", space="PSUM", bufs=1))
        # psum tags: "a" scores/QK/FFN1 [128,512]x2; "c" ctx [128,512]x2;
        # "b" V/attn-out/FFN2 [128,1024]x2 (768 used).  2+2+4 banks = 8.

        def rhs_x(nf, hi):
            """feature-major rhs [128, 512] for sequence-half nf, chunk hi."""
            return xT[:, 4 * nf:4 * nf + 4, hi, :]

        # ---------------- helpers ----------------
        def ln_store(src_ap, res_ap, dst, tcid, last=False, out_f32=None):
            """dst = layernorm(src + res); also writes bf16 copy to xtok[tcid]
            unless last (then DMAs fp32 to out_f32)."""
            s1 = small.tile([P, 1], f32, tag="s1")
            nc.vector.scalar_tensor_tensor(
                out=dst[:], in0=src_ap, scalar=0.0, in1=res_ap,
                op0=OP.add, op1=OP.add, accum_out=s1[:])
            u = small.tile([P, 1], f32, tag="u")
            nc.vector.tensor_scalar(out=u[:], in0=s1[:], scalar1=1.0 / H,
                                    scalar2=None, op0=OP.mult)
            junk = small.tile([P, H], f32, tag="junk", bufs=2)
            s2 = small.tile([P, 1], f32, tag="s2")
            nc.vector.scalar_tensor_tensor(
                out=junk[:], in0=dst[:], scalar=u[:], in1=dst[:],
                op0=OP.subtract, op1=OP.mult, accum_out=s2[:])
            sd = small.tile([P, 1], f32, tag="sd")
            nc.scalar.activation(sd[:], s2[:], AF.Sqrt, bias=eps_t[:], scale=1.0 / H)
            rstd = small.tile([P, 1], f32, tag="rstd")
            nc.vector.reciprocal(rstd[:], sd[:])
            nc.vector.tensor_scalar(out=dst[:], in0=dst[:], scalar1=u[:],
                                    scalar2=rstd[:], op0=OP.subtract, op1=OP.mult)
            if last:
                nc.scalar.dma_start(out_f32, dst[:])
            else:
                nc.scalar.copy(xtok[tcid][:], dst[:])

        def transpose_t(t):
            """xtok[t] (token-major bf16) -> xT[:, t] via DMA xbar.

            On the sync HWDGE queue; weight loads go through the gpsimd
            SWDGE queue so a transpose waiting on its xtok copy never
            head-of-line-blocks weight prefetch (queues are FIFO)."""
            nc.sync.dma_start_transpose(xT[:, t], xtok[t][:])

        # ---- embedding: gather + add + LN ----
        with ExitStack() as emb_scope:
            ep = emb_scope.enter_context(tc.tile_pool(name="emb", bufs=1))
            wg = ep.tile([P, TC, H], bf16, name="wg")
            tg = ep.tile([P, TC, H], bf16, name="tg")
            nc.sync.dma_start(wg[:], wrows.ap().rearrange("(c p) h -> p c h", p=P))
            nc.sync.dma_start(tg[:], trows.ap().rearrange("(c p) h -> p c h", p=P))
            pos = ep.tile([P, SC, H], f32, name="pos")
            nc.sync.dma_start(pos[:], pemb.ap().rearrange("(c p) h -> p c h", p=P))
            for t in range(TC):
                tmp = ep.tile([P, H], f32, tag="etmp", bufs=2, name="etmp")
                nc.vector.tensor_tensor(out=tmp[:], in0=tg[:, t],
                                        in1=pos[:, t % SC], op=OP.add)
                ln_store(wg[:, t], tmp[:], curA[t], t)
                transpose_t(t)
            tap("emb", curA)

        # ---- transformer layers ----
        for l in range(layers):
            with ExitStack() as ls:
                wp = ls.enter_context(tc.tile_pool(name=f"bias{l}", bufs=1))
                bq_t = wp.tile([P, HC], f32, name=f"bq{l}")
                bk_t = wp.tile([P, HC], f32, name=f"bk{l}")
                b1_t = wp.tile([P, IC], f32, name=f"b1{l}")
                nc.sync.dma_start(bq_t[:], dbq.ap()[l].rearrange("(c p) -> p c", p=P))
                nc.sync.dma_start(bk_t[:], dbk.ap()[l].rearrange("(c p) -> p c", p=P))
                nc.sync.dma_start(b1_t[:], db1.ap()[l].rearrange("(c p) -> p c", p=P))
                brow_t = wp.tile([1, 2 * H], bf16, name=f"brow{l}")
                nc.sync.dma_start(brow_t[:], dbrow.ap()[l])

                # single rotating pool for all weight chunks of this layer
                wpool = ls.enter_context(tc.tile_pool(name=f"w{l}", bufs=27))

                def wload(dW, r0, c0, cols=H):
                    # sync HWDGE: a buffer-wait here must not block gpsimd's
                    # FIFO (partition-broadcasts sit on the attention path)
                    w = wpool.tile([P, H], bf16, tag="wc")
                    nc.sync.dma_start(w[:, :cols],
                                      dW.ap()[l, r0:r0 + P, c0:c0 + cols])
                    return w

                cur, nxt = (curA, curB)

                with ExitStack() as attn_scope:
                    ap_ = attn_scope.enter_context(
                        tc.tile_pool(name=f"attn{l}", bufs=1))
                    QT = [ap_.tile([P, NT], bf16, name=f"QT{l}_{h}") for h in range(HC)]
                    KT = [ap_.tile([P, NT], bf16, name=f"KT{l}_{h}") for h in range(HC)]
                    # V with an appended ones column per head (65 cols/head)
                    Vt = [ap_.tile([P, NH, DH + 1], bf16, name=f"V{l}_{t}")
                          for t in range(TC)]
                    ctxT = [ap_.tile([P, NT], bf16, name=f"cT{l}_{h}") for h in range(HC)]

                    wv_c = [wload(dWv, h * P, 0) for h in range(HC)]
                    wq_c = [wload(dWq, h * P, 0) for h in range(HC)]
                    wk_c = [wload(dWk, h * P, 0) for h in range(HC)]
                    wo_c = [wload(dWo, h * P, 0) for h in range(HC)]
                    # W1 loads issued here (not at FFN emission) so the SWDGE
                    # dispatches aren't queued behind attention's broadcasts
                    w1_c = [[wload(dW1, hi * P, j * H) for j in range(IC // HC)]
                            for hi in range(HC)]

                    def qk_group(dstT, wch, bt, ho, nf):
                        ps = psum.tile([P, S], f32, tag="a", bufs=2, name="psqk")
                        for hi in range(HC):
                            nc.tensor.matmul(
                                ps[:], lhsT=wch[hi][:, ho * P:(ho + 1) * P],
                                rhs=rhs_x(nf, hi),
                                start=(hi == 0), stop=(hi == HC - 1))
                        nc.vector.tensor_scalar(
                            out=dstT[ho][:, nf * S:(nf + 1) * S], in0=ps[:],
                            scalar1=bt[:, ho:ho + 1], scalar2=None, op0=OP.add)

                    def v_group(t):
                        pv = psum.tile([P, 2 * S], f32, tag="b", bufs=2, name="psv")
                        for nf, n0, nn in ((0, 0, S), (1, S, H - S)):
                            for hi in range(HC):
                                nc.tensor.matmul(
                                    pv[:, n0:n0 + nn],
                                    lhsT=xT[:, t, hi, :],
                                    rhs=wv_c[hi][:, n0:n0 + nn],
                                    start=(hi == 0), stop=(hi == HC - 1))
                        nc.vector.tensor_copy(
                            Vt[t][:, :, 0:DH],
                            pv[:, 0:H].rearrange("p (h d) -> p h d", h=NH))
                        nc.vector.memset(Vt[t][:, :, DH:DH + 1], 1.0)

                    # --- attention unit pieces (seq s, head hd) ---
                    ex_tiles = {}

                    def sc_unit(s, hd):
                        """scoresT + exp for one (seq, head): 4 kc tiles."""
                        hc, po = hd // 2, (hd % 2) * DH
                        exs = []
                        for kc in range(SC):
                            ps = psum.tile([P, S], f32, tag="a", bufs=2, name="pss")
                            nc.tensor.matmul(
                                ps[:],
                                lhsT=KT[hc][po:po + DH,
                                            s * S + kc * P:s * S + (kc + 1) * P],
                                rhs=QT[hc][po:po + DH, s * S:(s + 1) * S],
                                start=True, stop=True, tile_position=(po, 0))
                            ex = ap_.tile([P, S], bf16, tag="ex", bufs=14, name="ex")
                            if with_mask:
                                nc.scalar.activation(
                                    ex[:], ps[:], AF.Exp,
                                    bias=extm_sb[:, s * SC + kc:s * SC + kc + 1],
                                    scale=INV_SQRT_DH)
                            else:
                                nc.scalar.activation(ex[:], ps[:], AF.Exp,
                                                     scale=INV_SQRT_DH)
                            exs.append(ex)
                        ex_tiles[(s, hd)] = exs

                    def ctx_unit(s, hd):
                        """ctx + denominator + normalize for one (seq, head)."""
                        hc, po = hd // 2, (hd % 2) * DH
                        exs = ex_tiles.pop((s, hd))
                        cx = psum.tile([P, S], f32, tag="c", bufs=2, name="cx")
                        for kc in range(SC):
                            nc.tensor.matmul(
                                cx[0:DH + 1, :],
                                lhsT=Vt[s * SC + kc][:, hd, :],
                                rhs=exs[kc][:],
                                start=(kc == 0), stop=(kc == SC - 1))
                        row = small.tile([1, S], f32, tag="row", bufs=4)
                        nc.vector.tensor_copy(row[:], cx[DH:DH + 1, :])
                        rinv = small.tile([1, S], f32, tag="rinv", bufs=4)
                        # ~51 ULP is plenty (feeds bf16 math); sums of
                        # positive exps can't hit the undefined edge cases
                        nc.vector.reciprocal_approx_fast(out=rinv[:], in_=row[:])
                        rb = small.tile([DH, S], f32, tag="rb", bufs=3)
                        nc.gpsimd.partition_broadcast(rb[:], rinv[:], channels=DH)
                        nc.vector.tensor_tensor(
                            out=ctxT[hc][po:po + DH, s * S:(s + 1) * S],
                            in0=cx[0:DH, :], in1=rb[:], op=OP.mult)

                    def ao_group(t):
                        """attn-out projection + residual + LN1 for chunk t."""
                        po_ = psum.tile([P, 2 * S], f32, tag="b", bufs=2, name="pao")
                        for nf, n0, nn in ((0, 0, S), (1, S, H - S)):
                            for hi in range(HC):
                                nc.tensor.matmul(
                                    po_[:, n0:n0 + nn],
                                    lhsT=ctxT[hi][:, t * P:(t + 1) * P],
                                    rhs=wo_c[hi][:, n0:n0 + nn],
                                    start=(hi == 0),
                                    stop=(hi == HC - 1 and not with_brow))
                            if with_brow:
                                nc.tensor.matmul(po_[:, n0:n0 + nn], lhsT=ones1[:],
                                                 rhs=brow_t[0:1, n0:n0 + nn],
                                                 start=False, stop=True)
                        ln_store(po_[:, 0:H], cur[t][:], nxt[t], t)
                        transpose_t(t)

                    # --- emission schedule ---
                    # Only nf0 (seq0) xT chunks are ready at layer start (the
                    # previous layer's t4..7 LN2 transposes land a few us in),
                    # so everything nf1-dependent is deferred into the seq1
                    # stream.  V(seq0) first so seq0 ctx units can interleave
                    # with the QK/scores stream (ctx lags scores by 2 units:
                    # at most 4 score units = 16 ex tiles in flight).
                    for t in range(SC):
                        v_group(t)
                    for hc in range(HC):
                        qk_group(QT, wq_c, bq_t, hc, 0)
                        qk_group(KT, wk_c, bk_t, hc, 0)
                        if hc > 0:
                            ctx_unit(0, 2 * hc - 2)
                            ctx_unit(0, 2 * hc - 1)
                        sc_unit(0, 2 * hc)
                        sc_unit(0, 2 * hc + 1)
                    # seq1: nf1 QK groups + V(seq1) spread into the scores
                    # pipeline, ctx lags scores by 2
                    ctx_unit(0, NH - 2)
                    ctx_unit(0, NH - 1)
                    for h in range(NH):
                        if h % 2 == 0:
                            qk_group(QT, wq_c, bq_t, h // 2, 1)
                            qk_group(KT, wk_c, bk_t, h // 2, 1)
                        sc_unit(1, h)
                        if h < 2:
                            v_group(SC + 2 * h)
                            v_group(SC + 2 * h + 1)
                        if h >= 2:
                            ctx_unit(1, h - 2)
                        if h >= NH - SC:
                            ao_group(h - (NH - SC))  # t0..3 (seq0 ctx done)
                    # tail: remaining ctx + seq1 attn-out/LN1
                    ctx_unit(1, NH - 2)
                    ctx_unit(1, NH - 1)
                    if l == 0:
                        tap("QT", QT)
                        tap("KT", KT)
                        tap("ctxT", ctxT)
                    for t in range(SC, TC):
                        ao_group(t)
                    if l == 0:
                        tap("ln1", [nxt[t] for t in range(TC)])

                # --- FFN ---
                with ExitStack() as ffn_scope:
                    fp_ = ffn_scope.enter_context(
                        tc.tile_pool(name=f"ffn{l}", bufs=1))
                    gT = [fp_.tile([P, NT], bf16, name=f"gT{l}_{i}") for i in range(IC)]

                    # nf-outer: the 24 nf0 groups cover the latency of the
                    # seq1 LN1 chains + transposes that nf1 depends on
                    for nf in range(2):
                        for i_ in range(IC):
                            ps = psum.tile([P, S], f32, tag="a", bufs=2, name="psf")
                            for hi in range(HC):
                                w1t = w1_c[hi][(i_ * P) // H]
                                c0 = (i_ * P) % H
                                nc.tensor.matmul(
                                    ps[:], lhsT=w1t[:, c0:c0 + P],
                                    rhs=rhs_x(nf, hi),
                                    start=(hi == 0), stop=(hi == HC - 1))
                            nc.scalar.activation(
                                gT[i_][:, nf * S:(nf + 1) * S], ps[:], AF.Gelu,
                                bias=b1_t[:, i_:i_ + 1], scale=1.0)
                    if l == 0:
                        tap("gT", gT)

                    w2_c = [wload(dW2, i_ * P, 0) for i_ in range(IC)]
                    for t in range(TC):
                        pf = psum.tile([P, 2 * S], f32, tag="b", bufs=2, name="pf")
                        for i_ in range(IC):
                            for nf, n0, nn in ((0, 0, S), (1, S, H - S)):
                                nc.tensor.matmul(
                                    pf[:, n0:n0 + nn],
                                    lhsT=gT[i_][:, t * P:(t + 1) * P],
                                    rhs=w2_c[i_][:, n0:n0 + nn],
                                    start=(i_ == 0),
                                    stop=(i_ == IC - 1 and not with_brow))
                        if with_brow:
                            for nf, n0, nn in ((0, 0, S), (1, S, H - S)):
                                nc.tensor.matmul(pf[:, n0:n0 + nn], lhsT=ones1[:],
                                                 rhs=brow_t[0:1, H + n0:H + n0 + nn],
                                                 start=False, stop=True)
                        last = (l == layers - 1)
                        ln_store(pf[:, 0:H], nxt[t][:], cur[t], t, last=last,
                                 out_f32=out.ap()[t * P:(t + 1) * P, :])
                        if not last:
                            transpose_t(t)

    nc.compile()
    return nc


# --------------------------------------------------------------------------
# host side
# --------------------------------------------------------------------------

def prep_shared(inputs):
    sh = {}
    sh["wemb_bf"] = inputs["word_emb"].astype(_BF16)
    sh["temb_bf"] = inputs["type_emb"].astype(_BF16)
    sh["pemb"] = inputs["pos_emb"].astype(np.float32)
    for k in ("Wq", "Wk", "Wv", "Wo", "W1", "W2"):
        sh[k] = inputs[k].astype(_BF16)
    sh["bq"] = inputs["bq"].astype(np.float32)
    sh["bk"] = inputs["bk"].astype(np.float32)
    sh["b1"] = inputs["b1"].astype(np.float32)
    # brow rows: [bv @ Wo + bo, b2]  (bv folded through the out-projection)
    bvWo = np.einsum("lh,lho->lo", inputs["bv"].astype(np.float64),
                     inputs["Wo"].astype(np.float64)).astype(np.float32)
    browo = (bvWo + inputs["bo"]).astype(np.float32)
    sh["brow"] = np.concatenate([browo, inputs["b2"]],
                                axis=1)[:, None, :].astype(_BF16)
    return sh


def core_inputs(inputs, sh, c):
    ids = np.asarray(inputs["input_ids"]).astype(np.int64)
    tts = np.asarray(inputs["token_type_ids"]).astype(np.int64)
    am = np.asarray(inputs["attention_mask"]).astype(np.float32)
    b0 = c * B_LOC
    m = {k: v for k, v in sh.items() if k not in ("wemb_bf", "temb_bf")}
    m["wrows"] = np.ascontiguousarray(sh["wemb_bf"][ids[b0:b0 + B_LOC].reshape(-1)])
    m["trows"] = np.ascontiguousarray(sh["temb_bf"][tts[b0:b0 + B_LOC].reshape(-1)])
    # [P, B_LOC*SC] per-partition mask columns
    em = ((1.0 - am[b0:b0 + B_LOC]) * -10000.0).reshape(B_LOC * SC, P)
    m["extmc"] = np.ascontiguousarray(em.T).astype(np.float32)
    return m


_NC_CACHE = {}


def flags_for(inputs):
    with_mask = not np.all(np.asarray(inputs["attention_mask"]) == 1.0)
    with_brow = bool(np.any(np.asarray(inputs["bo"])) or
                     np.any(np.asarray(inputs["b2"])) or
                     np.any(np.asarray(inputs["bv"])))
    return with_mask, with_brow


def get_nc(layers=L, with_mask=False, with_brow=False):
    key = (layers, with_mask, with_brow)
    if key not in _NC_CACHE:
        _NC_CACHE[key] = build(layers, with_mask=with_mask, with_brow=with_brow)
    return _NC_CACHE[key]


def run(inputs, layers=L):
    from concourse.bass_utils import run_bass_kernel_spmd
    inputs = {k: np.asarray(v) for k, v in inputs.items()}
    wm, wb = flags_for(inputs)
    nc = get_nc(layers, wm, wb)
    sh = prep_shared(inputs)
    in_maps = [core_inputs(inputs, sh, c) for c in range(NCORES)]
    res = run_bass_kernel_spmd(nc, in_maps, core_ids=list(range(NCORES)))
    outs = [res.results[c]["out"].reshape(B_LOC, S, H) for c in range(NCORES)]
    return np.concatenate(outs, axis=0).astype(np.float32)


def kernel(**inputs):
    return run(inputs)


# revision 22
# speedup vs baseline: 1.3556x; 1.0639x over previous
"""BERT-base forward on 8 Trainium2 NeuronCores.

Strategy: pure data parallelism over the batch (B=16 -> 2 sequences per
core), weights replicated, zero collectives.

v6 redesign (from v5's trace: ACT 52%/2.4ms + DVE 36%/1.66ms serialized
against PE 85%/3.9ms, 29% of the span HAM-throttled at half clock):

* k-major attention: scores are computed transposed ([k, q]) so the
  softmax probabilities feed the ctx matmul directly -- the 384 PE
  transpose-matmuls + 96 psum copies per layer of v5 are gone.  The
  softmax denominator comes for free as row 64 of the ctx matmul
  (V carries an appended ones column, M=65); normalization happens
  after ctx via DVE-recip -> gpsimd partition-broadcast -> DVE mult.
* token->feature layout changes (xtok -> xT) run on the DMA xbar
  transpose engine instead of PE matmul + ACT copy.
* PSUM evacuations moved from ACT to DVE; ACT keeps exp/gelu/sqrt and
  the bf16 xtok copies only.
* 1/sqrt(DH) folded into the exp scale; bv folded into the attn-out
  row bias host-side (brow_o = bv @ Wo + bo); mask folded into the
  exp per-partition bias (k-major makes the mask per-partition).
* single rotating weight-chunk pool ([128,768] bf16 x 24 bufs) streams
  Wq/Wk/Wv/Wo/W1/W2 and prefetches across phases.

Layout conventions per core (P=128 partitions):
  tokens NT=1024 (2 seqs x 512), token chunk t in [0,8)
  token-major  [128 tokens, H]   - residual stream, layernorm
  feature-major xT [128, t, hc, 128] - matmul lhsT/rhs operands
  matmul computes out = lhsT.T @ rhs (contraction along partitions)
"""

import numpy as np
import ml_dtypes

V, H, L, NH, I, S = 30522, 768, 12, 12, 3072, 512
B_FULL, NCORES, B_LOC = 16, 8, 2
DH = H // NH                      # 64
P = 128
NT = B_LOC * S                    # 1024 tokens per core
TC = NT // P                      # 8 token chunks
HC = H // P                       # 6 feature chunks
IC = I // P                       # 24 ffn chunks
SC = S // P                       # 4 chunks per sequence
EPS = 1e-12
INV_SQRT_DH = 1.0 / 8.0

_BF16 = ml_dtypes.bfloat16


# --------------------------------------------------------------------------
# device kernel builder
# --------------------------------------------------------------------------

def build(layers=L, taps=None, with_mask=False, with_brow=False):
    import concourse.bass as bass
    import concourse.mybir as mybir
    import concourse.tile as tile
    from concourse import bacc
    from contextlib import ExitStack

    dt = mybir.dt
    AF = mybir.ActivationFunctionType
    OP = mybir.AluOpType

    nc = bacc.Bacc("TRN2", target_bir_lowering=False, debug=False,
                   num_devices=NCORES)

    # ---- DRAM inputs (per core) ----
    wrows = nc.dram_tensor("wrows", [NT, H], dt.bfloat16, kind="ExternalInput")
    trows = nc.dram_tensor("trows", [NT, H], dt.bfloat16, kind="ExternalInput")
    pemb = nc.dram_tensor("pemb", [S, H], dt.float32, kind="ExternalInput")
    # mask as per-partition columns: extmc[p, s*SC+kc] = mask bias of
    # k-token s*512 + kc*128 + p (k-major scores make the mask per-partition)
    extmc = nc.dram_tensor("extmc", [P, B_LOC * SC], dt.float32,
                           kind="ExternalInput")
    dWq = nc.dram_tensor("Wq", [L, H, H], dt.bfloat16, kind="ExternalInput")
    dWk = nc.dram_tensor("Wk", [L, H, H], dt.bfloat16, kind="ExternalInput")
    dWv = nc.dram_tensor("Wv", [L, H, H], dt.bfloat16, kind="ExternalInput")
    dWo = nc.dram_tensor("Wo", [L, H, H], dt.bfloat16, kind="ExternalInput")
    dW1 = nc.dram_tensor("W1", [L, H, I], dt.bfloat16, kind="ExternalInput")
    dW2 = nc.dram_tensor("W2", [L, I, H], dt.bfloat16, kind="ExternalInput")
    dbq = nc.dram_tensor("bq", [L, H], dt.float32, kind="ExternalInput")
    dbk = nc.dram_tensor("bk", [L, H], dt.float32, kind="ExternalInput")
    db1 = nc.dram_tensor("b1", [L, I], dt.float32, kind="ExternalInput")
    # free-dim biases (added via K=1 rank-1 matmuls): rows [bv@Wo+bo, b2]
    dbrow = nc.dram_tensor("brow", [L, 1, 2 * H], dt.bfloat16, kind="ExternalInput")
    out = nc.dram_tensor("out", [NT, H], dt.float32, kind="ExternalOutput")

    f32, bf16 = dt.float32, dt.bfloat16

    def tap(name, tiles):
        if taps is None:
            return
        sh0 = list(tiles[0].shape)
        d = nc.dram_tensor(f"tap_{name}", [len(tiles)] + sh0,
                           tiles[0].dtype, kind="ExternalOutput")
        for i, t in enumerate(tiles):
            nc.sync.dma_start(d.ap()[i], t[:])
        taps[name] = d

    with tile.TileContext(nc) as tc_, ExitStack() as top:
        tc = tc_

        # ---- constants & persistent activation tiles ----
        pers = top.enter_context(tc.tile_pool(name="pers", bufs=1))
        ones1 = pers.tile([1, P], bf16, name="ones1")
        nc.vector.memset(ones1[:], 1.0)
        eps_t = pers.tile([P, 1], f32, name="eps_t")
        nc.vector.memset(eps_t[:], EPS)
        extm_sb = pers.tile([P, B_LOC * SC], f32, name="extm_sb")
        nc.sync.dma_start(extm_sb[:], extmc.ap())

        curA = [pers.tile([P, H], f32, name=f"curA{t}") for t in range(TC)]
        curB = [pers.tile([P, H], f32, name=f"curB{t}") for t in range(TC)]
        xtok = [pers.tile([P, H], bf16, name=f"xtok{t}") for t in range(TC)]
        # feature-major activations: xT[p, t, hc, j] = x[token t*128+j,
        # feature hc*128+p], written by DMA xbar transpose per t-chunk
        xT = pers.tile([P, TC, HC, P], bf16, name="xT")

        small = top.enter_context(tc.tile_pool(name="small", bufs=6))
        psum = top.enter_context(tc.tile_pool(name="psum", space="PSUM", bufs=1))
        # psum tags: "a" scores/QK/FFN1 [128,512]x2; "c" ctx [128,512]x2;
        # "b" V/attn-out/FFN2 [128,1024]x2 (768 used).  2+2+4 banks = 8.

        def rhs_x(nf, hi):
            """feature-major rhs [128, 512] for sequence-half nf, chunk hi."""
            return xT[:, 4 * nf:4 * nf + 4, hi, :]

        # ---------------- helpers ----------------
        def ln_store(src_ap, res_ap, dst, tcid, last=False, out_f32=None):
            """dst = layernorm(src + res); also writes bf16 copy to xtok[tcid]
            unless last (then DMAs fp32 to out_f32)."""
            s1 = small.tile([P, 1], f32, tag="s1")
            nc.vector.scalar_tensor_tensor(
                out=dst[:], in0=src_ap, scalar=0.0, in1=res_ap,
                op0=OP.add, op1=OP.add, accum_out=s1[:])
            u = small.tile([P, 1], f32, tag="u")
            nc.vector.tensor_scalar(out=u[:], in0=s1[:], scalar1=1.0 / H,
                                    scalar2=None, op0=OP.mult)
            junk = small.tile([P, H], f32, tag="junk", bufs=2)
            s2 = small.tile([P, 1], f32, tag="s2")
            nc.vector.scalar_tensor_tensor(
                out=junk[:], in0=dst[:], scalar=u[:], in1=dst[:],
                op0=OP.subtract, op1=OP.mult, accum_out=s2[:])
            sd = small.tile([P, 1], f32, tag="sd")
            nc.scalar.activation(sd[:], s2[:], AF.Sqrt, bias=eps_t[:], scale=1.0 / H)
            rstd = small.tile([P, 1], f32, tag="rstd")
            nc.vector.reciprocal(rstd[:], sd[:])
            nc.vector.tensor_scalar(out=dst[:], in0=dst[:], scalar1=u[:],
                                    scalar2=rstd[:], op0=OP.subtract, op1=OP.mult)
            if last:
                nc.scalar.dma_start(out_f32, dst[:])
            else:
                # DVE, not ACT: a Copy on ACT thrashes the activation table
                # against Exp/Gelu (1.3us reload each way)
                nc.vector.tensor_copy(xtok[tcid][:], dst[:])

        def transpose_t(t):
            """xtok[t] (token-major bf16) -> xT[:, t] via DMA xbar.

            On the sync HWDGE queue; weight loads go through the gpsimd
            SWDGE queue so a transpose waiting on its xtok copy never
            head-of-line-blocks weight prefetch (queues are FIFO)."""
            nc.sync.dma_start_transpose(xT[:, t], xtok[t][:])

        # ---- embedding: gather + add + LN ----
        with ExitStack() as emb_scope:
            ep = emb_scope.enter_context(tc.tile_pool(name="emb", bufs=1))
            wg = ep.tile([P, TC, H], bf16, name="wg")
            tg = ep.tile([P, TC, H], bf16, name="tg")
            nc.sync.dma_start(wg[:], wrows.ap().rearrange("(c p) h -> p c h", p=P))
            nc.sync.dma_start(tg[:], trows.ap().rearrange("(c p) h -> p c h", p=P))
            pos = ep.tile([P, SC, H], f32, name="pos")
            nc.sync.dma_start(pos[:], pemb.ap().rearrange("(c p) h -> p c h", p=P))
            for t in range(TC):
                tmp = ep.tile([P, H], f32, tag="etmp", bufs=2, name="etmp")
                nc.vector.tensor_tensor(out=tmp[:], in0=tg[:, t],
                                        in1=pos[:, t % SC], op=OP.add)
                ln_store(wg[:, t], tmp[:], curA[t], t)
                transpose_t(t)
            tap("emb", curA)

        # ---- transformer layers ----
        for l in range(layers):
            with ExitStack() as ls:
                wp = ls.enter_context(tc.tile_pool(name=f"bias{l}", bufs=1))
                bq_t = wp.tile([P, HC], f32, name=f"bq{l}")
                bk_t = wp.tile([P, HC], f32, name=f"bk{l}")
                b1_t = wp.tile([P, IC], f32, name=f"b1{l}")
                nc.sync.dma_start(bq_t[:], dbq.ap()[l].rearrange("(c p) -> p c", p=P))
                nc.sync.dma_start(bk_t[:], dbk.ap()[l].rearrange("(c p) -> p c", p=P))
                nc.sync.dma_start(b1_t[:], db1.ap()[l].rearrange("(c p) -> p c", p=P))
                brow_t = wp.tile([1, 2 * H], bf16, name=f"brow{l}")
                nc.sync.dma_start(brow_t[:], dbrow.ap()[l])

                # single rotating pool for all weight chunks of this layer
                wpool = ls.enter_context(tc.tile_pool(name=f"w{l}", bufs=27))

                def wload(dW, r0, c0, cols=H):
                    # sync HWDGE: a buffer-wait here must not block gpsimd's
                    # FIFO (partition-broadcasts sit on the attention path)
                    w = wpool.tile([P, H], bf16, tag="wc")
                    nc.sync.dma_start(w[:, :cols],
                                      dW.ap()[l, r0:r0 + P, c0:c0 + cols])
                    return w

                cur, nxt = (curA, curB)

                with ExitStack() as attn_scope:
                    ap_ = attn_scope.enter_context(
                        tc.tile_pool(name=f"attn{l}", bufs=1))
                    QT = [ap_.tile([P, NT], bf16, name=f"QT{l}_{h}") for h in range(HC)]
                    KT = [ap_.tile([P, NT], bf16, name=f"KT{l}_{h}") for h in range(HC)]
                    # V with an appended ones column per head (65 cols/head)
                    Vt = [ap_.tile([P, NH, DH + 1], bf16, name=f"V{l}_{t}")
                          for t in range(TC)]
                    ctxT = [ap_.tile([P, NT], bf16, name=f"cT{l}_{h}") for h in range(HC)]

                    wv_c = [wload(dWv, h * P, 0) for h in range(HC)]
                    wq_c = [wload(dWq, h * P, 0) for h in range(HC)]
                    wk_c = [wload(dWk, h * P, 0) for h in range(HC)]
                    wo_c = [wload(dWo, h * P, 0) for h in range(HC)]
                    # W1 loads issued here (not at FFN emission) so the SWDGE
                    # dispatches aren't queued behind attention's broadcasts
                    w1_c = [[wload(dW1, hi * P, j * H) for j in range(IC // HC)]
                            for hi in range(HC)]

                    def qk_group(dstT, wch, bt, ho, nf):
                        # "b" tag: keeps the "a" rotation exclusive to scores
                        # during attention (scores stall on exp otherwise)
                        ps = psum.tile([P, S], f32, tag="b", bufs=2, name="psqk")
                        for hi in range(HC):
                            nc.tensor.matmul(
                                ps[:], lhsT=wch[hi][:, ho * P:(ho + 1) * P],
                                rhs=rhs_x(nf, hi),
                                start=(hi == 0), stop=(hi == HC - 1))
                        nc.vector.tensor_scalar(
                            out=dstT[ho][:, nf * S:(nf + 1) * S], in0=ps[:],
                            scalar1=bt[:, ho:ho + 1], scalar2=None, op0=OP.add)

                    def v_group(t):
                        pv = psum.tile([P, 2 * S], f32, tag="b", bufs=2, name="psv")
                        for nf, n0, nn in ((0, 0, S), (1, S, H - S)):
                            for hi in range(HC):
                                nc.tensor.matmul(
                                    pv[:, n0:n0 + nn],
                                    lhsT=xT[:, t, hi, :],
                                    rhs=wv_c[hi][:, n0:n0 + nn],
                                    start=(hi == 0), stop=(hi == HC - 1))
                        nc.vector.tensor_copy(
                            Vt[t][:, :, 0:DH],
                            pv[:, 0:H].rearrange("p (h d) -> p h d", h=NH))
                        nc.vector.memset(Vt[t][:, :, DH:DH + 1], 1.0)

                    # --- attention unit pieces (seq s, head hd) ---
                    ex_tiles = {}

                    def sc_unit(s, hd):
                        """scoresT + exp for one (seq, head): 4 kc tiles."""
                        hc, po = hd // 2, (hd % 2) * DH
                        exs = []
                        for kc in range(SC):
                            ps = psum.tile([P, S], f32, tag="a", bufs=2, name="pss")
                            nc.tensor.matmul(
                                ps[:],
                                lhsT=KT[hc][po:po + DH,
                                            s * S + kc * P:s * S + (kc + 1) * P],
                                rhs=QT[hc][po:po + DH, s * S:(s + 1) * S],
                                start=True, stop=True, tile_position=(po, 0))
                            ex = ap_.tile([P, S], bf16, tag="ex", bufs=14, name="ex")
                            if with_mask:
                                nc.scalar.activation(
                                    ex[:], ps[:], AF.Exp,
                                    bias=extm_sb[:, s * SC + kc:s * SC + kc + 1],
                                    scale=INV_SQRT_DH)
                            else:
                                nc.scalar.activation(ex[:], ps[:], AF.Exp,
                                                     scale=INV_SQRT_DH)
                            exs.append(ex)
                        ex_tiles[(s, hd)] = exs

                    def ctx_unit(s, hd):
                        """ctx + denominator + normalize for one (seq, head)."""
                        hc, po = hd // 2, (hd % 2) * DH
                        exs = ex_tiles.pop((s, hd))
                        cx = psum.tile([P, S], f32, tag="c", bufs=2, name="cx")
                        for kc in range(SC):
                            nc.tensor.matmul(
                                cx[0:DH + 1, :],
                                lhsT=Vt[s * SC + kc][:, hd, :],
                                rhs=exs[kc][:],
                                start=(kc == 0), stop=(kc == SC - 1))
                        row = small.tile([1, S], f32, tag="row", bufs=4)
                        nc.vector.tensor_copy(row[:], cx[DH:DH + 1, :])
                        rinv = small.tile([1, S], f32, tag="rinv", bufs=4)
                        # ~51 ULP is plenty (feeds bf16 math); sums of
                        # positive exps can't hit the undefined edge cases
                        nc.vector.reciprocal_approx_fast(out=rinv[:], in_=row[:])
                        rb = small.tile([DH, S], f32, tag="rb", bufs=3)
                        nc.gpsimd.partition_broadcast(rb[:], rinv[:], channels=DH)
                        nc.vector.tensor_tensor(
                            out=ctxT[hc][po:po + DH, s * S:(s + 1) * S],
                            in0=cx[0:DH, :], in1=rb[:], op=OP.mult)

                    def ao_group(t):
                        """attn-out projection + residual + LN1 for chunk t."""
                        po_ = psum.tile([P, 2 * S], f32, tag="b", bufs=2, name="pao")
                        for nf, n0, nn in ((0, 0, S), (1, S, H - S)):
                            for hi in range(HC):
                                nc.tensor.matmul(
                                    po_[:, n0:n0 + nn],
                                    lhsT=ctxT[hi][:, t * P:(t + 1) * P],
                                    rhs=wo_c[hi][:, n0:n0 + nn],
                                    start=(hi == 0),
                                    stop=(hi == HC - 1 and not with_brow))
                            if with_brow:
                                nc.tensor.matmul(po_[:, n0:n0 + nn], lhsT=ones1[:],
                                                 rhs=brow_t[0:1, n0:n0 + nn],
                                                 start=False, stop=True)
                        ln_store(po_[:, 0:H], cur[t][:], nxt[t], t)
                        transpose_t(t)

                    # --- emission schedule ---
                    # Only nf0 (seq0) xT chunks are ready at layer start (the
                    # previous layer's t4..7 LN2 transposes land a few us in),
                    # so everything nf1-dependent is deferred into the seq1
                    # stream.  V(seq0) first so seq0 ctx units can interleave
                    # with the QK/scores stream (ctx lags scores by 2 units:
                    # at most 4 score units = 16 ex tiles in flight).
                    for t in range(SC):
                        v_group(t)
                    for hc in range(HC):
                        qk_group(QT, wq_c, bq_t, hc, 0)
                        qk_group(KT, wk_c, bk_t, hc, 0)
                        if hc > 0:
                            ctx_unit(0, 2 * hc - 2)
                            ctx_unit(0, 2 * hc - 1)
                        sc_unit(0, 2 * hc)
                        sc_unit(0, 2 * hc + 1)
                    # seq1: nf1 QK groups + V(seq1) spread into the scores
                    # pipeline, ctx lags scores by 2
                    ctx_unit(0, NH - 2)
                    ctx_unit(0, NH - 1)
                    for h in range(NH):
                        if h % 2 == 0:
                            qk_group(QT, wq_c, bq_t, h // 2, 1)
                            qk_group(KT, wk_c, bk_t, h // 2, 1)
                        sc_unit(1, h)
                        if h < 2:
                            v_group(SC + 2 * h)
                            v_group(SC + 2 * h + 1)
                        if h >= 2:
                            ctx_unit(1, h - 2)
                        if h >= NH - SC:
                            ao_group(h - (NH - SC))  # t0..3 (seq0 ctx done)
                    # tail: remaining ctx + seq1 attn-out/LN1
                    ctx_unit(1, NH - 2)
                    ctx_unit(1, NH - 1)
                    if l == 0:
                        tap("QT", QT)
                        tap("KT", KT)
                        tap("ctxT", ctxT)
                    for t in range(SC, TC):
                        ao_group(t)
                    if l == 0:
                        tap("ln1", [nxt[t] for t in range(TC)])

                # --- FFN ---
                with ExitStack() as ffn_scope:
                    fp_ = ffn_scope.enter_context(
                        tc.tile_pool(name=f"ffn{l}", bufs=1))
                    gT = [fp_.tile([P, NT], bf16, name=f"gT{l}_{i}") for i in range(IC)]

                    # nf-outer: the 24 nf0 groups cover the latency of the
                    # seq1 LN1 chains + transposes that nf1 depends on
                    for nf in range(2):
                        for i_ in range(IC):
                            ps = psum.tile([P, S], f32, tag="a", bufs=2, name="psf")
                            for hi in range(HC):
                                w1t = w1_c[hi][(i_ * P) // H]
                                c0 = (i_ * P) % H
                                nc.tensor.matmul(
                                    ps[:], lhsT=w1t[:, c0:c0 + P],
                                    rhs=rhs_x(nf, hi),
                                    start=(hi == 0), stop=(hi == HC - 1))
                            nc.scalar.activation(
                                gT[i_][:, nf * S:(nf + 1) * S], ps[:], AF.Gelu,
                                bias=b1_t[:, i_:i_ + 1], scale=1.0)
                    if l == 0:
                        tap("gT", gT)

                    w2_c = [wload(dW2, i_ * P, 0) for i_ in range(IC)]
                    for t in range(TC):
                        pf = psum.tile([P, 2 * S], f32, tag="b", bufs=2, name="pf")
                        for i_ in range(IC):
                            for nf, n0, nn in ((0, 0, S), (1, S, H - S)):
                                nc.tensor.matmul(
                                    pf[:, n0:n0 + nn],
                                    lhsT=gT[i_][:, t * P:(t + 1) * P],
                                    rhs=w2_c[i_][:, n0:n0 + nn],
                                    start=(i_ == 0),
                                    stop=(i_ == IC - 1 and not with_brow))
                        if with_brow:
                            for nf, n0, nn in ((0, 0, S), (1, S, H - S)):
                                nc.tensor.matmul(pf[:, n0:n0 + nn], lhsT=ones1[:],
                                                 rhs=brow_t[0:1, H + n0:H + n0 + nn],
                                                 start=False, stop=True)
                        last = (l == layers - 1)
                        ln_store(pf[:, 0:H], nxt[t][:], cur[t], t, last=last,
                                 out_f32=out.ap()[t * P:(t + 1) * P, :])
                        if not last:
                            transpose_t(t)

    nc.compile()
    return nc


# --------------------------------------------------------------------------
# host side
# --------------------------------------------------------------------------

def prep_shared(inputs):
    sh = {}
    sh["wemb_bf"] = inputs["word_emb"].astype(_BF16)
    sh["temb_bf"] = inputs["type_emb"].astype(_BF16)
    sh["pemb"] = inputs["pos_emb"].astype(np.float32)
    for k in ("Wq", "Wk", "Wv", "Wo", "W1", "W2"):
        sh[k] = inputs[k].astype(_BF16)
    sh["bq"] = inputs["bq"].astype(np.float32)
    sh["bk"] = inputs["bk"].astype(np.float32)
    sh["b1"] = inputs["b1"].astype(np.float32)
    # brow rows: [bv @ Wo + bo, b2]  (bv folded through the out-projection)
    bvWo = np.einsum("lh,lho->lo", inputs["bv"].astype(np.float64),
                     inputs["Wo"].astype(np.float64)).astype(np.float32)
    browo = (bvWo + inputs["bo"]).astype(np.float32)
    sh["brow"] = np.concatenate([browo, inputs["b2"]],
                                axis=1)[:, None, :].astype(_BF16)
    return sh


def core_inputs(inputs, sh, c):
    ids = np.asarray(inputs["input_ids"]).astype(np.int64)
    tts = np.asarray(inputs["token_type_ids"]).astype(np.int64)
    am = np.asarray(inputs["attention_mask"]).astype(np.float32)
    b0 = c * B_LOC
    m = {k: v for k, v in sh.items() if k not in ("wemb_bf", "temb_bf")}
    m["wrows"] = np.ascontiguousarray(sh["wemb_bf"][ids[b0:b0 + B_LOC].reshape(-1)])
    m["trows"] = np.ascontiguousarray(sh["temb_bf"][tts[b0:b0 + B_LOC].reshape(-1)])
    # [P, B_LOC*SC] per-partition mask columns
    em = ((1.0 - am[b0:b0 + B_LOC]) * -10000.0).reshape(B_LOC * SC, P)
    m["extmc"] = np.ascontiguousarray(em.T).astype(np.float32)
    return m


_NC_CACHE = {}


def flags_for(inputs):
    with_mask = not np.all(np.asarray(inputs["attention_mask"]) == 1.0)
    with_brow = bool(np.any(np.asarray(inputs["bo"])) or
                     np.any(np.asarray(inputs["b2"])) or
                     np.any(np.asarray(inputs["bv"])))
    return with_mask, with_brow


def get_nc(layers=L, with_mask=False, with_brow=False):
    key = (layers, with_mask, with_brow)
    if key not in _NC_CACHE:
        _NC_CACHE[key] = build(layers, with_mask=with_mask, with_brow=with_brow)
    return _NC_CACHE[key]


def run(inputs, layers=L):
    from concourse.bass_utils import run_bass_kernel_spmd
    inputs = {k: np.asarray(v) for k, v in inputs.items()}
    wm, wb = flags_for(inputs)
    nc = get_nc(layers, wm, wb)
    sh = prep_shared(inputs)
    in_maps = [core_inputs(inputs, sh, c) for c in range(NCORES)]
    res = run_bass_kernel_spmd(nc, in_maps, core_ids=list(range(NCORES)))
    outs = [res.results[c]["out"].reshape(B_LOC, S, H) for c in range(NCORES)]
    return np.concatenate(outs, axis=0).astype(np.float32)


def kernel(**inputs):
    return run(inputs)
